# revision 1
# baseline (speedup 1.0000x reference)
"""Causal self-attention (B=4, T=2048, C=1024, NH=16) on 8 TRN2 NeuronCores.

Sharding (per spec hint): tensor-parallel over heads x data-parallel over batch.
Core i handles batch b = i//2 and head-group g = i%2 (8 heads each).
  - c_attn column-parallel: each core computes q,k,v for its 8 heads.
  - attention: fully local per core (its heads, its batch element).
  - c_proj row-parallel: each core computes a partial (T,C) output from its
    512 features; a 2-core ReduceScatter over pairs [[0,1],[2,3],[4,5],[6,7]]
    sums the partials, each core keeping half the rows. Host concatenates.

Device algorithm (per core), all matmuls bf16 with fp32 PSUM accumulation:
  xT (C,T) staged transposed by host.
  qT = wq^T @ xT, kT = wk^T @ xT   (feature-major, 4 chunks of 128)
  v  = x @ wv                      (token-major) + ones column per head
  per head pair (2fc, 2fc+1), per q-block Q (512 wide):
    s^T[kchunk] = kT_h^T @ qT_h    (K=64 contraction, row-tiled pair -> concurrent)
    p = exp(0.125 * s^T)  (ScalarE, bf16 out); causal-zeroed on GpSimd for
        diagonal chunks; fully-masked chunks skipped entirely.
    o^T[65,512] += v_aug_h^T @ p   (v_aug has a ones column -> row 64 = softmax
        denominators, fused into the same matmul)
    yT_h = o^T[0:64] * (1/o^T[64])  (PE K=1 broadcast of the reciprocal row)
  partial[T-block] = yT^T @ wp + 0.5*b_proj ; ReduceScatter(add) over the pair.
"""

import sys

if "/opt/trn_rl_repo" not in sys.path:
    sys.path.insert(0, "/opt/trn_rl_repo")

import numpy as np
import ml_dtypes

import concourse.bass as bass
import concourse.bacc as bacc
import concourse.mybir as mybir
import concourse.tile as tile
from concourse.bass import ts, ds
from concourse.bass_utils import run_bass_kernel_spmd

BF16 = ml_dtypes.bfloat16
N_CORES = 8
B, T, C = 4, 2048, 1024
NH, HS = 16, 64
H_LOC = NH // 2        # heads per core
F = H_LOC * HS         # 512 local qkv features
NFC = F // 128         # 4 feature chunks (one head pair each)
NKC = T // 128         # 16 key chunks
NQ = T // 512          # 4 query blocks
NCOL = C // 512        # 2 output column blocks
REPLICA_GROUPS = [[0, 1], [2, 3], [4, 5], [6, 7]]

FP32 = mybir.dt.float32
BF = mybir.dt.bfloat16


def _build_nc():
    # Bacc (not plain Bass): its compile() pipeline runs
    # generate_event_semaphores, which splits sync waits so no instruction
    # carries more than the hardware allows (walrus rejects >1 otherwise).
    nc = bacc.Bacc(None, target_bir_lowering=False, num_devices=N_CORES)

    xT = nc.dram_tensor("xT", [C, T], BF, kind="ExternalInput")
    wq = nc.dram_tensor("wq", [C, F], BF, kind="ExternalInput")
    wk = nc.dram_tensor("wk", [C, F], BF, kind="ExternalInput")
    wv = nc.dram_tensor("wv", [C, F], BF, kind="ExternalInput")
    bq = nc.dram_tensor("bq", [F], FP32, kind="ExternalInput")
    bk = nc.dram_tensor("bk", [F], FP32, kind="ExternalInput")
    bv = nc.dram_tensor("bv", [F], FP32, kind="ExternalInput")
    wp = nc.dram_tensor("wp", [F, C], BF, kind="ExternalInput")
    bp = nc.dram_tensor("bp", [C], FP32, kind="ExternalInput")
    out = nc.dram_tensor("out", [T // 2, C], FP32, kind="ExternalOutput")

    with tile.TileContext(nc) as tc:
        _body(tc, xT, wq, wk, wv, bq, bk, bv, wp, bp, out)
    nc.compile()
    return nc


def _body(tc, xT, wq, wk, wv, bq, bk, bv, wp, bp, out):
    nc = tc.nc
    import contextlib

    ctx = contextlib.ExitStack()
    with ctx:
        wpool = ctx.enter_context(tc.tile_pool(name="weights", bufs=1))
        apool = ctx.enter_context(tc.tile_pool(name="acts", bufs=1))
        ppool = ctx.enter_context(tc.tile_pool(name="ptiles", bufs=3))
        npool = ctx.enter_context(tc.tile_pool(name="norm", bufs=2))
        outp = ctx.enter_context(tc.tile_pool(name="outsb", bufs=3))
        # PSUM budget (8 banks): sAB [128,1024] x3 bufs = 6, oA/oB 1 bank each = 2
        ps_s = ctx.enter_context(tc.tile_pool(name="ps_s", bufs=3, space="PSUM"))
        ps_o = ctx.enter_context(tc.tile_pool(name="ps_o", bufs=1, space="PSUM"))
        dpool = ctx.enter_context(tc.tile_pool(name="dram", bufs=1, space="DRAM"))

        # ---- stage inputs into SBUF ----
        x_sb = wpool.tile([128, C // 128, T], BF)
        nc.sync.dma_start(out=x_sb, in_=xT.rearrange("(ko p) t -> p ko t", p=128))
        wq_sb = wpool.tile([128, C // 128, F], BF)
        nc.sync.dma_start(out=wq_sb, in_=wq.rearrange("(ko p) f -> p ko f", p=128))
        wk_sb = wpool.tile([128, C // 128, F], BF)
        nc.sync.dma_start(out=wk_sb, in_=wk.rearrange("(ko p) f -> p ko f", p=128))
        wv_sb = wpool.tile([128, C // 128, F], BF)
        nc.sync.dma_start(out=wv_sb, in_=wv.rearrange("(ko p) f -> p ko f", p=128))
        wp_sb = wpool.tile([128, NFC, C], BF)
        nc.sync.dma_start(out=wp_sb, in_=wp.rearrange("(ko p) n -> p ko n", p=128))

        bq_sb = wpool.tile([128, NFC], FP32)
        nc.sync.dma_start(out=bq_sb, in_=bq.rearrange("(fo p) -> p fo", p=128))
        bk_sb = wpool.tile([128, NFC], FP32)
        nc.sync.dma_start(out=bk_sb, in_=bk.rearrange("(fo p) -> p fo", p=128))
        # broadcast biases across partitions (for token-major layouts)
        bv_bc = wpool.tile([128, F], FP32)
        nc.sync.dma_start(
            out=bv_bc,
            in_=bass.AP(tensor=bv.ap().tensor, offset=0, ap=[[0, 128], [1, F]]),
        )
        bp_bc = wpool.tile([128, C], FP32)
        nc.sync.dma_start(
            out=bp_bc,
            in_=bass.AP(tensor=bp.ap().tensor, offset=0, ap=[[0, 128], [1, C]]),
        )

        # ---- persistent activations ----
        qT_sb = apool.tile([128, NFC, T], BF)   # q, feature-major
        kT_sb = apool.tile([128, NFC, T], BF)   # k, feature-major
        # v token-major, 66-stride per head: cols 0:64 = v, col 64 = ones
        v_sb = apool.tile([128, NKC, H_LOC, 66], BF)
        nc.vector.memset(v_sb[:, :, :, 64:65], 1.0)
        yT_sb = apool.tile([128, NFC, T], BF)   # attention out, feature-major

        partial = dpool.tile([T, C], FP32)      # c_proj partial (pre-reduce)
        # per-Q-block ReduceScatter halves: core keeps [256,1024] per block
        rs_outs = [dpool.tile([256, C], FP32, name=f"rs_out{q}") for q in range(NQ)]

        KO = C // 128  # 8 contraction chunks for the projections

        # ---- qkv projection units (emitted piecemeal: half up front, the
        # rest interleaved into the exp-bound attention phase as PE filler) --
        def qk_unit(w_sb, b_sb, dst, fc, tq2):
            # one 1024-token span of q^T or k^T for head-pair chunk fc
            ps = ps_s.tile([128, 1024], FP32, tag="sAB")
            for kc in range(KO):
                for half in range(2):
                    nc.tensor.matmul(
                        ps[:, ts(half, 512)],
                        lhsT=w_sb[:, kc, ts(fc, 128)],
                        rhs=x_sb[:, kc, ds(tq2 * 1024 + half * 512, 512)],
                        start=(kc == 0),
                        stop=(kc == KO - 1),
                    )
            nc.scalar.activation(
                out=dst[:, fc, ts(tq2, 1024)],
                in_=ps,
                func=mybir.ActivationFunctionType.Identity,
                bias=b_sb[:, fc : fc + 1],
                scale=1.0,
            )

        def qk_unit_half(w_sb, b_sb, dst, fc, tq):
            # finer 512-token unit, for late interleave slots
            ps = ps_s.tile([128, 1024], FP32, tag="sAB")
            for kc in range(KO):
                nc.tensor.matmul(
                    ps[:, 0:512],
                    lhsT=w_sb[:, kc, ts(fc, 128)],
                    rhs=x_sb[:, kc, ts(tq, 512)],
                    start=(kc == 0),
                    stop=(kc == KO - 1),
                )
            nc.scalar.activation(
                out=dst[:, fc, ts(tq, 512)],
                in_=ps[:, 0:512],
                func=mybir.ActivationFunctionType.Identity,
                bias=b_sb[:, fc : fc + 1],
                scale=1.0,
            )

        def v_unit(tc_i):
            ps = ps_s.tile([128, 1024], FP32, tag="sAB")
            for kc in range(KO):
                nc.tensor.matmul(
                    ps[:, 0:512],
                    lhsT=x_sb[:, kc, ts(tc_i, 128)],
                    rhs=wv_sb[:, kc, :],
                    start=(kc == 0),
                    stop=(kc == KO - 1),
                )
            nc.vector.tensor_add(
                out=v_sb[:, tc_i, :, 0:64],
                in0=ps[:, 0:512].rearrange("p (h f) -> p h f", h=H_LOC),
                in1=bv_bc.rearrange("p (h f) -> p h f", h=H_LOC),
            )

        # prefix: everything attention blocks 0-1 need
        for fc in range(NFC):
            qk_unit(wq_sb, bq_sb, qT_sb, fc, 0)
            qk_unit(wk_sb, bk_sb, kT_sb, fc, 0)
        for tc_i in range(8):
            v_unit(tc_i)

        # filler units with emission deadlines:
        #   qT/kT second halves + v 8..11 -> before attention(2) reads them
        #   v 12..15 -> before attention(3)
        filler_by_block = {
            0: [lambda fc=fc: qk_unit(wq_sb, bq_sb, qT_sb, fc, 1) for fc in range(NFC)]
            + [lambda i=i: v_unit(i) for i in range(8, 12)],
            1: [lambda fc=fc: qk_unit(wk_sb, bk_sb, kT_sb, fc, 1) for fc in range(NFC)]
            + [lambda i=i: v_unit(i) for i in range(12, 16)],
        }

        # ---- phase 2+3: attention per q-block; c_proj pipelined one block behind
        def attention_block(Q, interleave=None, filler=()):
            filler = list(filler)
            nkc = 4 * Q + 4  # causal: only key chunks 0 .. 4Q+3 contribute
            LAG = 2  # AV matmuls trail the QK/exp pipeline by this many chunks
            for fc in range(NFC):  # head pair (2fc, 2fc+1)
                oA = ps_o.tile([65, 512], FP32, tag="oA")
                oB = ps_o.tile([65, 512], FP32, tag="oB")
                pbuf = {}

                def emit_av(kc, oA=oA, oB=oB, nkc=nkc, fc=fc):
                    pAB = pbuf.pop(kc)
                    nc.tensor.matmul(
                        oA,
                        lhsT=v_sb[:, kc, 2 * fc, 0:65],
                        rhs=pAB[:, 0:512],
                        start=(kc == 0),
                        stop=(kc == nkc - 1),
                    )
                    nc.tensor.matmul(
                        oB,
                        lhsT=v_sb[:, kc, 2 * fc + 1, 0:65],
                        rhs=pAB[:, ds(512, 512)],
                        start=(kc == 0),
                        stop=(kc == nkc - 1),
                    )

                for kc in range(nkc):
                    # heads A and B share one 2-bank psum tile: A in cols
                    # 0:512 (array rows 0:64), B in 512:1024 (rows 64:128);
                    # the row-tiled pair runs concurrently on the PE.
                    sAB = ps_s.tile([128, 1024], FP32, tag="sAB")
                    nc.tensor.matmul(
                        sAB[:, 0:512],
                        lhsT=kT_sb[0:64, fc, ts(kc, 128)],
                        rhs=qT_sb[0:64, fc, ts(Q, 512)],
                        start=True,
                        stop=True,
                        tile_position=(0, 0),
                    )
                    nc.tensor.matmul(
                        sAB[:, ds(512, 512)],
                        lhsT=kT_sb[64:128, fc, ts(kc, 128)],
                        rhs=qT_sb[64:128, fc, ts(Q, 512)],
                        start=True,
                        stop=True,
                        tile_position=(64, 0),
                    )
                    pAB = ppool.tile([128, 1024], BF, tag="pAB", bufs=4)
                    nc.scalar.activation(
                        out=pAB, in_=sAB, func=mybir.ActivationFunctionType.Exp,
                        scale=0.125,
                    )
                    if kc >= 4 * Q:
                        # crosses the causal boundary: zero exp of masked
                        # scores (k_global > q_global) for both head halves
                        nc.gpsimd.affine_select(
                            out=pAB.rearrange("p (h q) -> p h q", h=2),
                            in_=pAB.rearrange("p (h q) -> p h q", h=2),
                            compare_op=mybir.AluOpType.is_ge,
                            fill=0.0,
                            base=512 * Q - 128 * kc,
                            channel_multiplier=-1,
                            pattern=[[0, 2], [1, 512]],
                        )
                    pbuf[kc] = pAB
                    if kc >= LAG:
                        emit_av(kc - LAG)
                for kc in range(max(0, nkc - LAG), nkc):
                    emit_av(kc)
                # normalize: yT_h = oT[0:64] * (1 / oT[64]).
                # Everything off the TensorEngine queue: DVE approx
                # reciprocal + DMA partition-broadcast + DVE multiply.
                oA_sb = npool.tile([65, 512], FP32, tag="oAsb")
                oB_sb = npool.tile([65, 512], FP32, tag="oBsb")
                nc.vector.tensor_copy(out=oA_sb, in_=oA)
                nc.vector.tensor_copy(out=oB_sb, in_=oB)
                # custom-DVE reciprocal_approx_fast mishandles inputs at a
                # nonzero partition base -- stage row 64 down to partition 0
                rzA = npool.tile([1, 512], FP32, tag="rzA")
                rzB = npool.tile([1, 512], FP32, tag="rzB")
                nc.vector.tensor_copy(out=rzA, in_=oA_sb[64:65, :])
                nc.vector.tensor_copy(out=rzB, in_=oB_sb[64:65, :])
                rA = npool.tile([1, 512], FP32, tag="rA")
                rB = npool.tile([1, 512], FP32, tag="rB")
                nc.vector.reciprocal_approx_fast(out=rA, in_=rzA)
                nc.vector.reciprocal_approx_fast(out=rB, in_=rzB)
                # partition-broadcast via DRAM bounce (SBUF APs need nonzero
                # partition step; DRAM APs don't)
                rAd = dpool.tile([512], FP32, tag="rAd", bufs=2)
                rBd = dpool.tile([512], FP32, tag="rBd", bufs=2)
                nc.sync.dma_start(out=rAd[None, :], in_=rA)
                nc.sync.dma_start(out=rBd[None, :], in_=rB)
                bcA = npool.tile([64, 512], FP32, tag="bcA")
                bcB = npool.tile([64, 512], FP32, tag="bcB")
                nc.sync.dma_start(
                    out=bcA,
                    in_=bass.AP(tensor=rAd.tensor, offset=rAd.offset, ap=[[0, 64], [1, 512]]),
                )
                nc.sync.dma_start(
                    out=bcB,
                    in_=bass.AP(tensor=rBd.tensor, offset=rBd.offset, ap=[[0, 64], [1, 512]]),
                )
                # head A lives on partitions 0:64 of chunk fc
                nc.vector.tensor_mul(
                    out=yT_sb[0:64, fc, ts(Q, 512)], in0=oA_sb[0:64, :], in1=bcA
                )
                # head B must land on partitions 64:128 -> stage + DMA shift
                yB = npool.tile([64, 512], BF, tag="yB")
                nc.vector.tensor_mul(out=yB, in0=oB_sb[0:64, :], in1=bcB)
                nc.sync.dma_start(out=yT_sb[64:128, fc, ts(Q, 512)], in_=yB)

                if interleave is not None:
                    # slot one c_proj token-block of the previous q-block into
                    # the PE stream here -- the attention phase is exp-bound,
                    # so these matmuls ride in otherwise-idle PE slots
                    proj_tb(interleave, fc)
                # deferred qkv-projection units ride the same idle PE slots
                for _ in range(2):
                    if filler:
                        filler.pop(0)()

        def proj_tb(Q, tb):
            trow = Q * 4 + tb
            ps = ps_s.tile([128, 1024], FP32, tag="sAB")
            for ncol in range(NCOL):
                for fc in range(NFC):
                    nc.tensor.matmul(
                        ps[:, ts(ncol, 512)],
                        lhsT=yT_sb[:, fc, ts(trow, 128)],
                        rhs=wp_sb[:, fc, ts(ncol, 512)],
                        start=(fc == 0),
                        stop=(fc == NFC - 1),
                    )
            o_sb = outp.tile([128, 1024], FP32, tag="osb")
            nc.vector.tensor_add(out=o_sb, in0=ps, in1=bp_bc)
            nc.sync.dma_start(out=partial[ds(trow * 128, 128), :], in_=o_sb)

        def rs_block(Q):
            # reduce this 512-token block across the batch pair while later
            # blocks still compute; each core keeps 256 of the 512 rows.
            return nc.gpsimd.collective_compute(
                "ReduceScatter",
                mybir.AluOpType.add,
                replica_groups=REPLICA_GROUPS,
                ins=[partial[ds(Q * 512, 512), :]],
                outs=[rs_outs[Q][:]],
            )

        # software pipeline: block Q's c_proj matmuls interleave into the
        # exp-bound attention phase of block Q+1, one token-block per head
        # pair; its ReduceScatter launches right after.
        last_cc = None
        for Q in range(NQ):
            attention_block(
                Q,
                interleave=Q - 1 if Q > 0 else None,
                filler=filler_by_block.get(Q, ()),
            )
            if Q > 0:
                last_cc = rs_block(Q - 1)
        for tb in range(4):
            proj_tb(NQ - 1, tb)
        last_cc = rs_block(NQ - 1)
        # output copies last: by now RS(0..2) are long done; only the final
        # block's wait is real. The explicit dep pins these at the tail of
        # the DMA queue -- otherwise the scheduler slots them right after
        # their own RS, where their sem wait head-blocks mid-kernel DMAs.
        from concourse.tile import add_dep_helper

        for Q in range(NQ):
            cp = nc.sync.dma_start(
                out=out.ap()[ds(Q * 256, 256), :], in_=rs_outs[Q][:]
            )
            add_dep_helper(cp.ins, last_cc.ins, reason="pin out-copy to kernel tail")


_NC_CACHE = None


def _get_nc():
    global _NC_CACHE
    if _NC_CACHE is None:
        _NC_CACHE = _build_nc()
    return _NC_CACHE


def kernel(x, w_attn, b_attn, w_proj, b_proj):
    x = np.asarray(x)
    w_attn = np.asarray(w_attn)
    b_attn = np.asarray(b_attn)
    w_proj = np.asarray(w_proj)
    b_proj = np.asarray(b_proj)

    nc = _get_nc()

    in_maps = []
    for i in range(N_CORES):
        b, g = i // 2, i % 2
        cols = slice(g * F, (g + 1) * F)
        in_maps.append(
            {
                "xT": np.ascontiguousarray(x[b].T).astype(BF16),
                "wq": np.ascontiguousarray(w_attn[:, g * F : (g + 1) * F]).astype(BF16),
                "wk": np.ascontiguousarray(
                    w_attn[:, C + g * F : C + (g + 1) * F]
                ).astype(BF16),
                "wv": np.ascontiguousarray(
                    w_attn[:, 2 * C + g * F : 2 * C + (g + 1) * F]
                ).astype(BF16),
                "bq": np.ascontiguousarray(b_attn[g * F : (g + 1) * F]).astype(
                    np.float32
                ),
                "bk": np.ascontiguousarray(b_attn[C + g * F : C + (g + 1) * F]).astype(
                    np.float32
                ),
                "bv": np.ascontiguousarray(
                    b_attn[2 * C + g * F : 2 * C + (g + 1) * F]
                ).astype(np.float32),
                "wp": np.ascontiguousarray(w_proj[g * F : (g + 1) * F, :]).astype(BF16),
                "bp": (b_proj * 0.5).astype(np.float32),
            }
        )

    global _last_in_maps
    _last_in_maps = in_maps  # stashed for external profiling harnesses
    res = run_bass_kernel_spmd(nc, in_maps, core_ids=list(range(N_CORES)))

    # Each core's "out" holds NQ blocks of 256 rows: block Q is the core's
    # ReduceScatter half of token rows [Q*512, (Q+1)*512) -- rank 0 (even
    # core) the first 256, rank 1 (odd core) the last 256.
    out = np.empty((B, T, C), dtype=np.float32)
    for b in range(B):
        even = res.results[2 * b]["out"].reshape(NQ, 256, C)
        odd = res.results[2 * b + 1]["out"].reshape(NQ, 256, C)
        blocks = out[b].reshape(NQ, 2, 256, C)
        blocks[:, 0] = even
        blocks[:, 1] = odd
    return out



# revision 3
# speedup vs baseline: 1.0703x; 1.0703x over previous
"""Causal self-attention (B=4, T=2048, C=1024, NH=16) on 8 TRN2 NeuronCores.

Sharding (per spec hint): tensor-parallel over heads x data-parallel over batch.
Core i handles batch b = i//2 and head-group g = i%2 (8 heads each).
  - c_attn column-parallel: each core computes q,k,v for its 8 heads.
  - attention: fully local per core (its heads, its batch element).
  - c_proj row-parallel: each core computes a partial (T,C) output from its
    512 features; a 2-core ReduceScatter over pairs [[0,1],[2,3],[4,5],[6,7]]
    sums the partials. The RS runs at 128-token granularity (16 small ops
    pipelined behind the compute) and each 64-row result is copied to the
    output as soon as its RS lands -- no serial tail pile-up.

Device algorithm (per core), matmuls bf16 with fp32 PSUM accumulation:
  xT (C,T) staged transposed by host; inputs staged in dependency order so
  the first qkv matmuls start ~10us in.
  qT = wq^T @ xT, kT = wk^T @ xT   (feature-major, 512-token units)
  v  = x @ wv                      (token-major) + ones column per head
  per head pair (2fc, 2fc+1), per q-block Q (512 wide):
    s^T[kchunk] = kT_h^T @ qT_h    (K=64 contraction, row-tiled pair ->
        concurrent); columns trimmed to the causal range on diagonal chunks.
    p = exp(0.125 * s^T)  (ScalarE, bf16 out) over causal columns only; the
        128-wide diagonal triangle is zeroed by a DVE multiply with a
        precomputed triangular mask (GpSimd affine_select builds it once).
    o^T[65,W] += v_aug_h^T @ p     (ones column -> row 64 = softmax denom)
    r = approx_recip(denoms) on DVE; r is partition-broadcast on GpSimd
    (daisy chain, SBUF only -- no DMA round trip, no PSUM); y^T_h =
    o^T[0:64] * bc. The broadcast+multiplies are deferred into the next
    head-pair's score stream so their waits never head-block a queue.
  c_proj pipelined one q-block behind, one 128-token block per head-pair
  slot, interleaved into the exp-bound attention phase as PE filler; its
  ReduceScatter launches immediately, its output copy one slot later.
"""

import sys

if "/opt/trn_rl_repo" not in sys.path:
    sys.path.insert(0, "/opt/trn_rl_repo")

import numpy as np
import ml_dtypes

import concourse.bass as bass
import concourse.bacc as bacc
import concourse.mybir as mybir
import concourse.tile as tile
from concourse.bass import ts, ds
from concourse.bass_utils import run_bass_kernel_spmd

BF16 = ml_dtypes.bfloat16
N_CORES = 8
B, T, C = 4, 2048, 1024
NH, HS = 16, 64
H_LOC = NH // 2        # heads per core
F = H_LOC * HS         # 512 local qkv features
NFC = F // 128         # 4 feature chunks (one head pair each)
NKC = T // 128         # 16 key chunks
NQ = T // 512          # 4 query blocks
NCOL = C // 512        # 2 output column blocks
KO = C // 128          # 8 contraction chunks for the projections
NTB = T // 128         # 16 c_proj token blocks
REPLICA_GROUPS = [[0, 1], [2, 3], [4, 5], [6, 7]]

FP32 = mybir.dt.float32
BF = mybir.dt.bfloat16


def _build_nc():
    # Bacc (not plain Bass): its compile() pipeline runs
    # generate_event_semaphores, which splits sync waits so no instruction
    # carries more than the hardware allows (walrus rejects >1 otherwise).
    nc = bacc.Bacc(None, target_bir_lowering=False, num_devices=N_CORES)

    xT = nc.dram_tensor("xT", [C, T], BF, kind="ExternalInput")
    wq = nc.dram_tensor("wq", [C, F], BF, kind="ExternalInput")
    wk = nc.dram_tensor("wk", [C, F], BF, kind="ExternalInput")
    wv = nc.dram_tensor("wv", [C, F], BF, kind="ExternalInput")
    bq = nc.dram_tensor("bq", [F], FP32, kind="ExternalInput")
    bk = nc.dram_tensor("bk", [F], FP32, kind="ExternalInput")
    bv = nc.dram_tensor("bv", [F], FP32, kind="ExternalInput")
    wp = nc.dram_tensor("wp", [F, C], BF, kind="ExternalInput")
    bp = nc.dram_tensor("bp", [C], FP32, kind="ExternalInput")
    out = nc.dram_tensor("out", [T // 2, C], FP32, kind="ExternalOutput")

    with tile.TileContext(nc) as tc:
        _body(tc, xT, wq, wk, wv, bq, bk, bv, wp, bp, out)
    nc.compile()
    return nc


def _body(tc, xT, wq, wk, wv, bq, bk, bv, wp, bp, out):
    nc = tc.nc
    import contextlib

    ctx = contextlib.ExitStack()
    with ctx:
        wpool = ctx.enter_context(tc.tile_pool(name="weights", bufs=1))
        apool = ctx.enter_context(tc.tile_pool(name="acts", bufs=1))
        ppool = ctx.enter_context(tc.tile_pool(name="ptiles", bufs=3))
        npool = ctx.enter_context(tc.tile_pool(name="norm", bufs=2))
        outp = ctx.enter_context(tc.tile_pool(name="outsb", bufs=3))
        # PSUM budget (8 banks): sAB [128,1024] x3 bufs = 6, oA/oB 1 each = 2
        ps_s = ctx.enter_context(tc.tile_pool(name="ps_s", bufs=3, space="PSUM"))
        ps_o = ctx.enter_context(tc.tile_pool(name="ps_o", bufs=1, space="PSUM"))
        dpool = ctx.enter_context(tc.tile_pool(name="dram", bufs=1, space="DRAM"))

        # ---- activation-table preload: a tiny exp up front so the ~2.7us
        # ACT_TABLE_LOAD overlaps input staging instead of the first score.
        warm_in = wpool.tile([1, 16], FP32)
        warm_out = wpool.tile([1, 16], BF)
        nc.vector.memset(warm_in, 0.0)
        nc.scalar.activation(
            out=warm_out, in_=warm_in,
            func=mybir.ActivationFunctionType.Exp, scale=1.0,
        )

        # ---- stage inputs into SBUF (ordered so compute starts early) ----
        wq_sb = wpool.tile([128, KO, F], BF)
        nc.sync.dma_start(out=wq_sb, in_=wq.rearrange("(ko p) f -> p ko f", p=128))
        wk_sb = wpool.tile([128, KO, F], BF)
        nc.sync.dma_start(out=wk_sb, in_=wk.rearrange("(ko p) f -> p ko f", p=128))
        bq_sb = wpool.tile([128, NFC], FP32)
        nc.sync.dma_start(out=bq_sb, in_=bq.rearrange("(fo p) -> p fo", p=128))
        bk_sb = wpool.tile([128, NFC], FP32)
        nc.sync.dma_start(out=bk_sb, in_=bk.rearrange("(fo p) -> p fo", p=128))
        # x staged in two token halves so the first qk units start sooner
        x_sb = wpool.tile([128, KO, T], BF)
        x_re = xT.rearrange("(ko p) t -> p ko t", p=128)
        nc.sync.dma_start(out=x_sb[:, :, 0 : T // 2], in_=x_re[:, :, 0 : T // 2])
        nc.sync.dma_start(out=x_sb[:, :, T // 2 : T], in_=x_re[:, :, T // 2 : T])
        wv_sb = wpool.tile([128, KO, F], BF)
        nc.sync.dma_start(out=wv_sb, in_=wv.rearrange("(ko p) f -> p ko f", p=128))
        wp_sb = wpool.tile([128, NFC, C], BF)
        nc.sync.dma_start(out=wp_sb, in_=wp.rearrange("(ko p) n -> p ko n", p=128))
        # broadcast biases across partitions (for token-major layouts)
        bv_bc = wpool.tile([128, F], FP32)
        nc.sync.dma_start(
            out=bv_bc,
            in_=bass.AP(tensor=bv.ap().tensor, offset=0, ap=[[0, 128], [1, F]]),
        )
        bp_bc = wpool.tile([128, C], FP32)
        nc.sync.dma_start(
            out=bp_bc,
            in_=bass.AP(tensor=bp.ap().tensor, offset=0, ap=[[0, 128], [1, C]]),
        )

        # ---- constants ----
        # triangular causal mask for the 128-wide diagonal band:
        # tri[p, h, t] = 1 if t >= p else 0  (query-offset t vs key p)
        tri = wpool.tile([128, 2, 128], BF)
        nc.vector.memset(tri, 1.0)
        nc.gpsimd.affine_select(
            out=tri, in_=tri,
            compare_op=mybir.AluOpType.is_ge,
            fill=0.0, base=0, channel_multiplier=-1,
            pattern=[[0, 2], [1, 128]],
        )

        # ---- persistent activations ----
        qT_sb = apool.tile([128, NFC, T], BF)   # q, feature-major
        kT_sb = apool.tile([128, NFC, T], BF)   # k, feature-major
        # v token-major, 66-stride per head: cols 0:64 = v, col 64 = ones
        v_sb = apool.tile([128, NKC, H_LOC, 66], BF)
        nc.vector.memset(v_sb[:, :, :, 64:65], 1.0)
        yT_sb = apool.tile([128, NFC, T], BF)   # attention out, feature-major

        partial = dpool.tile([T, C], FP32)      # c_proj partial (pre-reduce)
        # per-token-block ReduceScatter halves: core keeps 64 of 128 rows
        rs_outs = [dpool.tile([64, C], FP32, name=f"rs_out{i}") for i in range(NTB)]

        # ---- qkv projection units (512-token granularity so they slot
        # finely into the attention phase as PE filler) ----
        def qk_unit(w_sb, b_sb, dst, fc, tq):
            ps = ps_s.tile([128, 1024], FP32, tag="sAB")
            for kc in range(KO):
                nc.tensor.matmul(
                    ps[:, 0:512],
                    lhsT=w_sb[:, kc, ts(fc, 128)],
                    rhs=x_sb[:, kc, ts(tq, 512)],
                    start=(kc == 0),
                    stop=(kc == KO - 1),
                )
            nc.scalar.activation(
                out=dst[:, fc, ts(tq, 512)],
                in_=ps[:, 0:512],
                func=mybir.ActivationFunctionType.Identity,
                bias=b_sb[:, fc : fc + 1],
                scale=1.0,
            )

        def v_unit(tc_i):
            ps = ps_s.tile([128, 1024], FP32, tag="sAB")
            for kc in range(KO):
                nc.tensor.matmul(
                    ps[:, 0:512],
                    lhsT=x_sb[:, kc, ts(tc_i, 128)],
                    rhs=wv_sb[:, kc, :],
                    start=(kc == 0),
                    stop=(kc == KO - 1),
                )
            nc.vector.tensor_add(
                out=v_sb[:, tc_i, :, 0:64],
                in0=ps[:, 0:512].rearrange("p (h f) -> p h f", h=H_LOC),
                in1=bv_bc.rearrange("p (h f) -> p h f", h=H_LOC),
            )

        # ---- c_proj token block + fine-grained ReduceScatter ----
        copies = []  # token-blocks whose RS output copy is still pending

        def proj_tb(Q, tb):
            trow = Q * 4 + tb
            ps = ps_s.tile([128, 1024], FP32, tag="sAB")
            for ncol in range(NCOL):
                for fc in range(NFC):
                    nc.tensor.matmul(
                        ps[:, ts(ncol, 512)],
                        lhsT=yT_sb[:, fc, ts(trow, 128)],
                        rhs=wp_sb[:, fc, ts(ncol, 512)],
                        start=(fc == 0),
                        stop=(fc == NFC - 1),
                    )
            o_sb = outp.tile([128, 1024], FP32, tag="osb")
            nc.vector.tensor_add(out=o_sb, in0=ps, in1=bp_bc)
            nc.sync.dma_start(out=partial[ds(trow * 128, 128), :], in_=o_sb)
            nc.gpsimd.collective_compute(
                "ReduceScatter",
                mybir.AluOpType.add,
                replica_groups=REPLICA_GROUPS,
                ins=[partial[ds(trow * 128, 128), :]],
                outs=[rs_outs[trow][:]],
            )
            copies.append(trow)

        def emit_copy(keep=1):
            # copy the oldest finished RS half to the output; keep the
            # newest `keep` pending so the copy never waits on a live RS
            while len(copies) > keep:
                trow = copies.pop(0)
                nc.sync.dma_start(
                    out=out.ap()[ds(trow * 64, 64), :], in_=rs_outs[trow][:]
                )

        # ---- attention ----
        pending = []  # deferred normalization phase-2 closures

        def flush_pending():
            while pending:
                pending.pop(0)()

        def attention_block(Q, pre_fc=None, slot_fns=None):
            pre_fc = pre_fc or {}
            slot_fns = slot_fns or {}
            nkc = 4 * Q + 4  # causal: only key chunks 0 .. 4Q+3 contribute
            LAG = 2  # AV matmuls trail the QK/exp pipeline by this many chunks
            for fc in range(NFC):  # head pair (2fc, 2fc+1)
                for u in pre_fc.get(fc, ()):
                    u()
                to = ps_o.tile([128, 512], FP32, tag="oA")
                tb_ = ps_o.tile([128, 512], FP32, tag="oB")
                pbuf = {}

                def emit_av(kc, to=to, tb_=tb_, nkc=nkc, fc=fc):
                    pAB, q0 = pbuf.pop(kc)
                    w = 512 - q0
                    nc.tensor.matmul(
                        to[0:65, ds(q0, w)],
                        lhsT=v_sb[:, kc, 2 * fc, 0:65],
                        rhs=pAB[:, ds(q0, w)],
                        start=(kc == 0),
                        stop=(kc == nkc - 1),
                    )
                    nc.tensor.matmul(
                        tb_[0:65, ds(q0, w)],
                        lhsT=v_sb[:, kc, 2 * fc + 1, 0:65],
                        rhs=pAB[:, ds(512 + q0, w)],
                        start=(kc == 0),
                        stop=(kc == nkc - 1),
                    )

                for kc in range(nkc):
                    j = kc - 4 * Q  # >= 0 on the diagonal band
                    q0 = 128 * j if j > 0 else 0
                    w = 512 - q0
                    # heads A and B share one 2-bank psum tile: A in cols
                    # 0:512 (array rows 0:64), B in 512:1024 (rows 64:128);
                    # the row-tiled pair runs concurrently on the PE.
                    sAB = ps_s.tile([128, 1024], FP32, tag="sAB")
                    nc.tensor.matmul(
                        sAB[:, ds(q0, w)],
                        lhsT=kT_sb[0:64, fc, ts(kc, 128)],
                        rhs=qT_sb[0:64, fc, ds(Q * 512 + q0, w)],
                        start=True,
                        stop=True,
                        tile_position=(0, 0),
                    )
                    nc.tensor.matmul(
                        sAB[:, ds(512 + q0, w)],
                        lhsT=kT_sb[64:128, fc, ts(kc, 128)],
                        rhs=qT_sb[64:128, fc, ds(Q * 512 + q0, w)],
                        start=True,
                        stop=True,
                        tile_position=(64, 0),
                    )
                    if kc == 2:
                        flush_pending()
                    pAB = ppool.tile([128, 1024], BF, tag="pAB", bufs=4)
                    pABh = pAB.rearrange("p (h q) -> p h q", h=2)
                    sABh = sAB.rearrange("p (h q) -> p h q", h=2)
                    # exp only the causal columns (columns < q0 are never
                    # read downstream: the AV rhs is trimmed to match)
                    nc.scalar.activation(
                        out=pABh[:, :, ds(q0, w)],
                        in_=sABh[:, :, ds(q0, w)],
                        func=mybir.ActivationFunctionType.Exp,
                        scale=0.125,
                    )
                    if j >= 0:
                        # zero the 128-wide causal triangle (DVE multiply
                        # with the precomputed mask)
                        nc.vector.tensor_mul(
                            out=pABh[:, :, ds(q0, 128)],
                            in0=pABh[:, :, ds(q0, 128)],
                            in1=tri,
                        )
                    pbuf[kc] = (pAB, q0)
                    if kc >= LAG:
                        emit_av(kc - LAG)
                for kc in range(max(0, nkc - LAG), nkc):
                    emit_av(kc)

                # normalization phase 1 (DVE): copy o out of PSUM (freeing
                # the banks), stage the denominator rows to partition 0,
                # approx-reciprocal.
                oA_sb = npool.tile([65, 512], FP32, tag="oAsb")
                oB_sb = npool.tile([65, 512], FP32, tag="oBsb")
                nc.vector.tensor_copy(out=oA_sb, in_=to[0:65, :])
                nc.vector.tensor_copy(out=oB_sb, in_=tb_[0:65, :])
                rz = npool.tile([1, 1024], FP32, tag="rz")
                nc.vector.tensor_copy(out=rz[:, 0:512], in_=oA_sb[64:65, :])
                nc.vector.tensor_copy(out=rz[:, 512:1024], in_=oB_sb[64:65, :])
                r = npool.tile([1, 1024], FP32, tag="r")
                nc.vector.reciprocal_approx_fast(out=r, in_=rz)

                # phase 2 (GpSimd broadcast + DVE multiplies) is deferred
                # into the next head-pair's score stream so its reciprocal
                # wait never head-blocks the DVE/GpSimd queues.
                def phase2(Q=Q, fc=fc, oA_sb=oA_sb, oB_sb=oB_sb, r=r):
                    # broadcast r from partition 0 to 64 partitions via the
                    # GpSimd daisy chain -- no DMA round trip, no PSUM
                    bc = npool.tile([64, 1024], FP32, tag="bc")
                    nc.gpsimd.partition_broadcast(out_ap=bc, in_ap=r, channels=64)
                    nc.vector.tensor_mul(
                        out=yT_sb[0:64, fc, ts(Q, 512)],
                        in0=oA_sb[0:64, :],
                        in1=bc[:, 0:512],
                    )
                    # head B must land on partitions 64:128 -> stage + DMA
                    yB = npool.tile([64, 512], BF, tag="yB")
                    nc.vector.tensor_mul(out=yB, in0=oB_sb[0:64, :], in1=bc[:, 512:1024])
                    nc.sync.dma_start(out=yT_sb[64:128, fc, ts(Q, 512)], in_=yB)

                pending.append(phase2)

                for fn in slot_fns.get(fc, ()):
                    fn()

        # ---- software-pipelined schedule ----
        # prefix: exactly what attention(0) fc0 needs
        qk_unit(wq_sb, bq_sb, qT_sb, 0, 0)
        qk_unit(wk_sb, bk_sb, kT_sb, 0, 0)
        for i in range(4):
            v_unit(i)

        def qkq(fc, tq):
            return lambda: qk_unit(wq_sb, bq_sb, qT_sb, fc, tq)

        def qkk(fc, tq):
            return lambda: qk_unit(wk_sb, bk_sb, kT_sb, fc, tq)

        attention_block(
            0,
            pre_fc={f: [qkq(f, 0), qkk(f, 0)] for f in (1, 2, 3)},
            slot_fns={f: [qkq(f, 1), qkk(f, 1), (lambda i=f: v_unit(4 + i))]
                      for f in range(4)},
        )
        attention_block(
            1,
            slot_fns={
                f: [qkq(f, 2), qkk(f, 2), (lambda i=f: v_unit(8 + i)),
                    (lambda i=f: proj_tb(0, i)), emit_copy]
                for f in range(4)
            },
        )
        attention_block(
            2,
            slot_fns={
                f: [qkq(f, 3), qkk(f, 3), (lambda i=f: v_unit(12 + i)),
                    (lambda i=f: proj_tb(1, i)), emit_copy]
                for f in range(4)
            },
        )
        attention_block(
            3,
            slot_fns={f: [(lambda i=f: proj_tb(2, i)), emit_copy]
                      for f in range(4)},
        )
        flush_pending()
        for tb in range(4):
            proj_tb(3, tb)
            emit_copy(keep=2)
        emit_copy(keep=0)


_NC_CACHE = None


def _get_nc():
    global _NC_CACHE
    if _NC_CACHE is None:
        _NC_CACHE = _build_nc()
    return _NC_CACHE


def kernel(x, w_attn, b_attn, w_proj, b_proj):
    x = np.asarray(x)
    w_attn = np.asarray(w_attn)
    b_attn = np.asarray(b_attn)
    w_proj = np.asarray(w_proj)
    b_proj = np.asarray(b_proj)

    nc = _get_nc()

    in_maps = []
    for i in range(N_CORES):
        b, g = i // 2, i % 2
        in_maps.append(
            {
                "xT": np.ascontiguousarray(x[b].T).astype(BF16),
                "wq": np.ascontiguousarray(w_attn[:, g * F : (g + 1) * F]).astype(BF16),
                "wk": np.ascontiguousarray(
                    w_attn[:, C + g * F : C + (g + 1) * F]
                ).astype(BF16),
                "wv": np.ascontiguousarray(
                    w_attn[:, 2 * C + g * F : 2 * C + (g + 1) * F]
                ).astype(BF16),
                "bq": np.ascontiguousarray(b_attn[g * F : (g + 1) * F]).astype(
                    np.float32
                ),
                "bk": np.ascontiguousarray(b_attn[C + g * F : C + (g + 1) * F]).astype(
                    np.float32
                ),
                "bv": np.ascontiguousarray(
                    b_attn[2 * C + g * F : 2 * C + (g + 1) * F]
                ).astype(np.float32),
                "wp": np.ascontiguousarray(w_proj[g * F : (g + 1) * F, :]).astype(BF16),
                "bp": (b_proj * 0.5).astype(np.float32),
            }
        )

    global _last_in_maps
    _last_in_maps = in_maps  # stashed for external profiling harnesses
    res = run_bass_kernel_spmd(nc, in_maps, core_ids=list(range(N_CORES)))

    # Each core's "out" holds NTB blocks of 64 rows: block i is the core's
    # ReduceScatter half of token rows [i*128, (i+1)*128) -- rank 0 (even
    # core) the first 64, rank 1 (odd core) the last 64.
    out = np.empty((B, T, C), dtype=np.float32)
    for b in range(B):
        even = res.results[2 * b]["out"].reshape(NTB, 64, C)
        odd = res.results[2 * b + 1]["out"].reshape(NTB, 64, C)
        blocks = out[b].reshape(NTB, 2, 64, C)
        blocks[:, 0] = even
        blocks[:, 1] = odd
    return out


# revision 5
# speedup vs baseline: 1.1452x; 1.0700x over previous
"""Causal self-attention (B=4, T=2048, C=1024, NH=16) on 8 TRN2 NeuronCores.

Sharding: tensor-parallel over heads x data-parallel over batch.
Core i handles batch b = i//2 and head-group g = i%2 (8 heads each).
  - c_attn column-parallel: each core computes q,k,v for its 8 heads.
  - attention: fully local per core (its heads, its batch element).
  - c_proj COLUMN-parallel: after attention, the pair [2b, 2b+1] exchanges
    normalized head outputs y (bf16) via one small AllGather per
    (q-block, head-pair); each core then computes out[:, my 512 columns] for
    ALL tokens using its wp column slice (the rank-dependence lives in the
    host-provided wp/bp inputs, so the device program is rank-independent).
    No ReduceScatter, no fp32 partial traffic, no output copies: c_proj
    results go straight to the output tensor.

Device algorithm (per core), matmuls bf16 with fp32 PSUM accumulation:
  xT (C,T) staged transposed by host; inputs staged in dependency order so
  the first qkv matmuls start ~9us in.
  qT = wq^T @ xT, kT = wk^T @ xT   (feature-major, 512-token units)
  v  = x @ wv                      (token-major) + ones column per head
  per head pair (2fc, 2fc+1), per q-block Q (512 wide):
    s^T[kchunk] = kT_h^T @ qT_h    (K=64 contraction, row-tiled pair ->
        concurrent); columns trimmed to the causal range on diagonal chunks.
    p = exp(0.125 * s^T)  (ScalarE, bf16 out) over causal columns only; the
        128-wide diagonal triangle is zeroed by a DVE multiply with a
        precomputed triangular mask (GpSimd affine_select builds it once).
    o^T[65,W] += v_aug_h^T @ p     (ones column -> row 64 = softmax denom)
    r = approx_recip(denoms) on DVE; r is partition-broadcast on GpSimd
    (daisy chain, SBUF only -- no DMA round trip, no PSUM); y_h = o^T * bc,
    staged to DRAM and AllGathered across the pair. The broadcast +
    multiplies + exchange are deferred into the next head-pair's score
    stream so their waits never head-block a queue.
  c_proj pipelined one q-block behind, one 128-token block per head-pair
  slot, interleaved into the exp-bound attention phase as PE filler; its
  contraction orders the last-exchanged feature chunks last so it can start
  before the final AllGather lands.
"""

import sys

if "/opt/trn_rl_repo" not in sys.path:
    sys.path.insert(0, "/opt/trn_rl_repo")

import numpy as np
import ml_dtypes

import concourse.bass as bass
import concourse.bacc as bacc
import concourse.mybir as mybir
import concourse.tile as tile
from concourse.bass import ts, ds
from concourse.bass_utils import run_bass_kernel_spmd

BF16 = ml_dtypes.bfloat16
N_CORES = 8
B, T, C = 4, 2048, 1024
NH, HS = 16, 64
H_LOC = NH // 2        # heads per core
F = H_LOC * HS         # 512 local qkv features
NFC = F // 128         # 4 feature chunks (one head pair each)
NKC = T // 128         # 16 key chunks
NQ = T // 512          # 4 query blocks
KO = C // 128          # 8 contraction chunks for c_proj (full features)
REPLICA_GROUPS = [[0, 1], [2, 3], [4, 5], [6, 7]]
# c_proj contraction order: chunks from the last-exchanged head pairs last,
# so the token-block matmuls can start before the final AllGather lands
KO_ORDER = [0, 4, 1, 5, 2, 6, 3, 7]

FP32 = mybir.dt.float32
BF = mybir.dt.bfloat16


def _build_nc():
    # Bacc (not plain Bass): its compile() pipeline runs
    # generate_event_semaphores, which splits sync waits so no instruction
    # carries more than the hardware allows (walrus rejects >1 otherwise).
    nc = bacc.Bacc(None, target_bir_lowering=False, num_devices=N_CORES)

    xT = nc.dram_tensor("xT", [C, T], BF, kind="ExternalInput")
    wq = nc.dram_tensor("wq", [C, F], BF, kind="ExternalInput")
    wk = nc.dram_tensor("wk", [C, F], BF, kind="ExternalInput")
    wv = nc.dram_tensor("wv", [C, F], BF, kind="ExternalInput")
    bq = nc.dram_tensor("bq", [F], FP32, kind="ExternalInput")
    bk = nc.dram_tensor("bk", [F], FP32, kind="ExternalInput")
    bv = nc.dram_tensor("bv", [F], FP32, kind="ExternalInput")
    wp = nc.dram_tensor("wp", [C, F], BF, kind="ExternalInput")  # col slice
    bp = nc.dram_tensor("bp", [F], FP32, kind="ExternalInput")   # col slice
    out = nc.dram_tensor("out", [T, F], FP32, kind="ExternalOutput")

    with tile.TileContext(nc) as tc:
        _body(tc, xT, wq, wk, wv, bq, bk, bv, wp, bp, out)
    nc.compile()
    return nc


def _body(tc, xT, wq, wk, wv, bq, bk, bv, wp, bp, out):
    nc = tc.nc
    import contextlib

    ctx = contextlib.ExitStack()
    with ctx:
        wpool = ctx.enter_context(tc.tile_pool(name="weights", bufs=1))
        apool = ctx.enter_context(tc.tile_pool(name="acts", bufs=1))
        ppool = ctx.enter_context(tc.tile_pool(name="ptiles", bufs=3))
        npool = ctx.enter_context(tc.tile_pool(name="norm", bufs=2))
        yfpool = ctx.enter_context(tc.tile_pool(name="yfull", bufs=2))
        outp = ctx.enter_context(tc.tile_pool(name="outsb", bufs=3))
        # PSUM budget (8 banks): sAB [128,1024] x3 bufs = 6, oA/oB 1 each = 2
        ps_s = ctx.enter_context(tc.tile_pool(name="ps_s", bufs=3, space="PSUM"))
        ps_o = ctx.enter_context(tc.tile_pool(name="ps_o", bufs=1, space="PSUM"))
        dpool = ctx.enter_context(tc.tile_pool(name="dram", bufs=1, space="DRAM"))

        # ---- activation-table preload: a tiny exp up front so the ~2.7us
        # ACT_TABLE_LOAD overlaps input staging instead of the first score.
        warm_in = wpool.tile([1, 16], FP32)
        warm_out = wpool.tile([1, 16], BF)
        nc.vector.memset(warm_in, 0.0)
        nc.scalar.activation(
            out=warm_out, in_=warm_in,
            func=mybir.ActivationFunctionType.Exp, scale=1.0,
        )

        # ---- stage inputs into SBUF (ordered so compute starts early) ----
        wq_sb = wpool.tile([128, KO, F], BF)
        nc.sync.dma_start(out=wq_sb, in_=wq.rearrange("(ko p) f -> p ko f", p=128))
        bq_sb = wpool.tile([128, NFC], FP32)
        nc.sync.dma_start(out=bq_sb, in_=bq.rearrange("(fo p) -> p fo", p=128))
        # x staged in two token halves so the first qk units start sooner
        x_sb = wpool.tile([128, KO, T], BF)
        x_re = xT.rearrange("(ko p) t -> p ko t", p=128)
        nc.sync.dma_start(out=x_sb[:, :, 0 : T // 2], in_=x_re[:, :, 0 : T // 2])
        wk_sb = wpool.tile([128, KO, F], BF)
        nc.sync.dma_start(out=wk_sb, in_=wk.rearrange("(ko p) f -> p ko f", p=128))
        bk_sb = wpool.tile([128, NFC], FP32)
        nc.sync.dma_start(out=bk_sb, in_=bk.rearrange("(fo p) -> p fo", p=128))
        wv_sb = wpool.tile([128, KO, F], BF)
        nc.sync.dma_start(out=wv_sb, in_=wv.rearrange("(ko p) f -> p ko f", p=128))
        nc.sync.dma_start(out=x_sb[:, :, T // 2 : T], in_=x_re[:, :, T // 2 : T])
        wp_sb = wpool.tile([128, KO, F], BF)
        nc.sync.dma_start(out=wp_sb, in_=wp.rearrange("(ko p) n -> p ko n", p=128))
        # broadcast biases across partitions (for token-major layouts)
        bv_bc = wpool.tile([128, F], FP32)
        nc.sync.dma_start(
            out=bv_bc,
            in_=bass.AP(tensor=bv.ap().tensor, offset=0, ap=[[0, 128], [1, F]]),
        )
        bp_bc = wpool.tile([128, F], FP32)
        nc.sync.dma_start(
            out=bp_bc,
            in_=bass.AP(tensor=bp.ap().tensor, offset=0, ap=[[0, 128], [1, F]]),
        )

        # ---- constants ----
        # triangular causal mask for the 128-wide diagonal band:
        # tri[p, h, t] = 1 if t >= p else 0  (query-offset t vs key p)
        tri = wpool.tile([128, 2, 128], BF)
        nc.vector.memset(tri, 1.0)
        nc.gpsimd.affine_select(
            out=tri, in_=tri,
            compare_op=mybir.AluOpType.is_ge,
            fill=0.0, base=0, channel_multiplier=-1,
            pattern=[[0, 2], [1, 128]],
        )

        # ---- persistent activations ----
        qT_sb = apool.tile([128, NFC, T], BF)   # q, feature-major
        kT_sb = apool.tile([128, NFC, T], BF)   # k, feature-major
        # v token-major, 66-stride per head: cols 0:64 = v, col 64 = ones
        v_sb = apool.tile([128, NKC, H_LOC, 66], BF)
        nc.vector.memset(v_sb[:, :, :, 64:65], 1.0)

        # per-(block, head-pair) AllGather staging. yd = our 128-feature
        # chunk of y^T for the block; ya[r] = rank r's chunk (rank 0 = head
        # group 0 = global feature chunk fc, rank 1 = chunk 4+fc).
        yd = [[dpool.tile([128, 512], BF, name=f"yd{q}_{f}") for f in range(NFC)]
              for q in range(NQ)]
        ya = [[dpool.tile([2, 128, 512], BF, name=f"ya{q}_{f}") for f in range(NFC)]
              for q in range(NQ)]
        yfs = {}  # Q -> gathered full-feature y^T [128, KO, 512] in SBUF

        # ---- qkv projection units (512-token granularity so they slot
        # finely into the attention phase as PE filler) ----
        def qk_unit(w_sb, b_sb, dst, fc, tq):
            ps = ps_s.tile([128, 1024], FP32, tag="sAB")
            for kc in range(KO):
                nc.tensor.matmul(
                    ps[:, 0:512],
                    lhsT=w_sb[:, kc, ts(fc, 128)],
                    rhs=x_sb[:, kc, ts(tq, 512)],
                    start=(kc == 0),
                    stop=(kc == KO - 1),
                )
            nc.scalar.activation(
                out=dst[:, fc, ts(tq, 512)],
                in_=ps[:, 0:512],
                func=mybir.ActivationFunctionType.Identity,
                bias=b_sb[:, fc : fc + 1],
                scale=1.0,
            )

        def v_unit(tc_i):
            ps = ps_s.tile([128, 1024], FP32, tag="sAB")
            for kc in range(KO):
                nc.tensor.matmul(
                    ps[:, 0:512],
                    lhsT=x_sb[:, kc, ts(tc_i, 128)],
                    rhs=wv_sb[:, kc, :],
                    start=(kc == 0),
                    stop=(kc == KO - 1),
                )
            nc.vector.tensor_add(
                out=v_sb[:, tc_i, :, 0:64],
                in0=ps[:, 0:512].rearrange("p (h f) -> p h f", h=H_LOC),
                in1=bv_bc.rearrange("p (h f) -> p h f", h=H_LOC),
            )

        # ---- c_proj: our 512 output columns for one 128-token block ----
        def proj_tb(Q, tb):
            yf = yfs[Q]
            ps = ps_s.tile([128, 1024], FP32, tag="sAB")
            for i, ko in enumerate(KO_ORDER):
                nc.tensor.matmul(
                    ps[:, 0:512],
                    lhsT=yf[:, ko, ts(tb, 128)],
                    rhs=wp_sb[:, ko, :],
                    start=(i == 0),
                    stop=(i == KO - 1),
                )
            o_sb = outp.tile([128, 512], FP32, tag="osb")
            nc.vector.tensor_add(out=o_sb, in0=ps[:, 0:512], in1=bp_bc)
            nc.sync.dma_start(
                out=out.ap()[ds(Q * 512 + tb * 128, 128), :], in_=o_sb
            )

        # ---- attention ----
        pending = []  # deferred normalization phase-2 closures

        def flush_pending():
            while pending:
                pending.pop(0)()

        def attention_block(Q, pre_fc=None, slot_fns=None):
            pre_fc = pre_fc or {}
            slot_fns = slot_fns or {}
            nkc = 4 * Q + 4  # causal: only key chunks 0 .. 4Q+3 contribute
            LAG = 2  # AV matmuls trail the QK/exp pipeline by this many chunks
            yf = yfpool.tile([128, KO, 512], BF, tag="yf")
            yfs[Q] = yf
            for fc in range(NFC):  # head pair (2fc, 2fc+1)
                for u in pre_fc.get(fc, ()):
                    u()
                to = ps_o.tile([128, 512], FP32, tag="oA")
                tb_ = ps_o.tile([128, 512], FP32, tag="oB")
                pbuf = {}

                def emit_av(kc, to=to, tb_=tb_, nkc=nkc, fc=fc):
                    pAB, q0 = pbuf.pop(kc)
                    w = 512 - q0
                    nc.tensor.matmul(
                        to[0:65, ds(q0, w)],
                        lhsT=v_sb[:, kc, 2 * fc, 0:65],
                        rhs=pAB[:, ds(q0, w)],
                        start=(kc == 0),
                        stop=(kc == nkc - 1),
                    )
                    nc.tensor.matmul(
                        tb_[0:65, ds(q0, w)],
                        lhsT=v_sb[:, kc, 2 * fc + 1, 0:65],
                        rhs=pAB[:, ds(512 + q0, w)],
                        start=(kc == 0),
                        stop=(kc == nkc - 1),
                    )

                for kc in range(nkc):
                    j = kc - 4 * Q  # >= 0 on the diagonal band
                    q0 = 128 * j if j > 0 else 0
                    w = 512 - q0
                    # heads A and B share one 2-bank psum tile: A in cols
                    # 0:512 (array rows 0:64), B in 512:1024 (rows 64:128);
                    # the row-tiled pair runs concurrently on the PE.
                    sAB = ps_s.tile([128, 1024], FP32, tag="sAB")
                    nc.tensor.matmul(
                        sAB[:, ds(q0, w)],
                        lhsT=kT_sb[0:64, fc, ts(kc, 128)],
                        rhs=qT_sb[0:64, fc, ds(Q * 512 + q0, w)],
                        start=True,
                        stop=True,
                        tile_position=(0, 0),
                    )
                    nc.tensor.matmul(
                        sAB[:, ds(512 + q0, w)],
                        lhsT=kT_sb[64:128, fc, ts(kc, 128)],
                        rhs=qT_sb[64:128, fc, ds(Q * 512 + q0, w)],
                        start=True,
                        stop=True,
                        tile_position=(64, 0),
                    )
                    if kc == 2:
                        flush_pending()
                    pAB = ppool.tile([128, 1024], BF, tag="pAB", bufs=4)
                    pABh = pAB.rearrange("p (h q) -> p h q", h=2)
                    sABh = sAB.rearrange("p (h q) -> p h q", h=2)
                    # exp only the causal columns (columns < q0 are never
                    # read downstream: the AV rhs is trimmed to match)
                    nc.scalar.activation(
                        out=pABh[:, :, ds(q0, w)],
                        in_=sABh[:, :, ds(q0, w)],
                        func=mybir.ActivationFunctionType.Exp,
                        scale=0.125,
                    )
                    if j >= 0:
                        # zero the 128-wide causal triangle (DVE multiply
                        # with the precomputed mask)
                        nc.vector.tensor_mul(
                            out=pABh[:, :, ds(q0, 128)],
                            in0=pABh[:, :, ds(q0, 128)],
                            in1=tri,
                        )
                    pbuf[kc] = (pAB, q0)
                    if kc >= LAG:
                        emit_av(kc - LAG)
                for kc in range(max(0, nkc - LAG), nkc):
                    emit_av(kc)

                # normalization phase 1 (DVE): copy o out of PSUM (freeing
                # the banks), stage the denominator rows to partition 0,
                # approx-reciprocal.
                oA_sb = npool.tile([65, 512], FP32, tag="oAsb")
                oB_sb = npool.tile([65, 512], FP32, tag="oBsb")
                nc.vector.tensor_copy(out=oA_sb, in_=to[0:65, :])
                nc.vector.tensor_copy(out=oB_sb, in_=tb_[0:65, :])
                rz = npool.tile([1, 1024], FP32, tag="rz")
                nc.vector.tensor_copy(out=rz[:, 0:512], in_=oA_sb[64:65, :])
                nc.vector.tensor_copy(out=rz[:, 512:1024], in_=oB_sb[64:65, :])
                r = npool.tile([1, 1024], FP32, tag="r")
                nc.vector.reciprocal_approx_fast(out=r, in_=rz)

                # phase 2 (GpSimd broadcast + DVE multiplies + exchange) is
                # deferred into the next head-pair's score stream so its
                # reciprocal wait never head-blocks the DVE/GpSimd queues.
                def phase2(Q=Q, fc=fc, oA_sb=oA_sb, oB_sb=oB_sb, r=r, yf=yf):
                    # broadcast r from partition 0 to 64 partitions via the
                    # GpSimd daisy chain -- no DMA round trip, no PSUM
                    bc = npool.tile([64, 1024], FP32, tag="bc")
                    nc.gpsimd.partition_broadcast(out_ap=bc, in_ap=r, channels=64)
                    ystA = npool.tile([64, 512], BF, tag="ystA")
                    ystB = npool.tile([64, 512], BF, tag="ystB")
                    nc.vector.tensor_mul(out=ystA, in0=oA_sb[0:64, :], in1=bc[:, 0:512])
                    nc.vector.tensor_mul(out=ystB, in0=oB_sb[0:64, :], in1=bc[:, 512:1024])
                    # stage our feature chunk to DRAM and exchange it
                    ydt = yd[Q][fc]
                    nc.sync.dma_start(out=ydt[ds(0, 64), :], in_=ystA)
                    nc.sync.dma_start(out=ydt[ds(64, 64), :], in_=ystB)
                    nc.gpsimd.collective_compute(
                        "AllGather",
                        mybir.AluOpType.bypass,
                        replica_groups=REPLICA_GROUPS,
                        ins=[ydt[:]],
                        outs=[ya[Q][fc][:]],
                    )
                    nc.sync.dma_start(out=yf[:, fc, :], in_=ya[Q][fc][0])
                    nc.sync.dma_start(out=yf[:, 4 + fc, :], in_=ya[Q][fc][1])

                pending.append(phase2)

                for fn in slot_fns.get(fc, ()):
                    fn()

        # ---- software-pipelined schedule ----
        # prefix: exactly what attention(0) fc0 needs
        qk_unit(wq_sb, bq_sb, qT_sb, 0, 0)
        qk_unit(wk_sb, bk_sb, kT_sb, 0, 0)
        for i in range(4):
            v_unit(i)

        def qkq(fc, tq):
            return lambda: qk_unit(wq_sb, bq_sb, qT_sb, fc, tq)

        def qkk(fc, tq):
            return lambda: qk_unit(wk_sb, bk_sb, kT_sb, fc, tq)

        attention_block(
            0,
            pre_fc={f: [qkq(f, 0), qkk(f, 0)] for f in (1, 2, 3)},
            slot_fns={f: [qkq(f, 1), qkk(f, 1), (lambda i=f: v_unit(4 + i))]
                      for f in range(4)},
        )
        attention_block(
            1,
            slot_fns={
                f: [qkq(f, 2), qkk(f, 2), (lambda i=f: v_unit(8 + i)),
                    (lambda i=f: proj_tb(0, i))]
                for f in range(4)
            },
        )
        attention_block(
            2,
            slot_fns={
                f: [qkq(f, 3), qkk(f, 3), (lambda i=f: v_unit(12 + i)),
                    (lambda i=f: proj_tb(1, i))]
                for f in range(4)
            },
        )
        attention_block(
            3,
            slot_fns={f: [(lambda i=f: proj_tb(2, i))] for f in range(4)},
        )
        flush_pending()
        for tb in range(4):
            proj_tb(3, tb)


_NC_CACHE = None


def _get_nc():
    global _NC_CACHE
    if _NC_CACHE is None:
        _NC_CACHE = _build_nc()
    return _NC_CACHE


def kernel(x, w_attn, b_attn, w_proj, b_proj):
    x = np.asarray(x)
    w_attn = np.asarray(w_attn)
    b_attn = np.asarray(b_attn)
    w_proj = np.asarray(w_proj)
    b_proj = np.asarray(b_proj)

    nc = _get_nc()

    in_maps = []
    for i in range(N_CORES):
        b, g = i // 2, i % 2
        in_maps.append(
            {
                "xT": np.ascontiguousarray(x[b].T).astype(BF16),
                "wq": np.ascontiguousarray(w_attn[:, g * F : (g + 1) * F]).astype(BF16),
                "wk": np.ascontiguousarray(
                    w_attn[:, C + g * F : C + (g + 1) * F]
                ).astype(BF16),
                "wv": np.ascontiguousarray(
                    w_attn[:, 2 * C + g * F : 2 * C + (g + 1) * F]
                ).astype(BF16),
                "bq": np.ascontiguousarray(b_attn[g * F : (g + 1) * F]).astype(
                    np.float32
                ),
                "bk": np.ascontiguousarray(b_attn[C + g * F : C + (g + 1) * F]).astype(
                    np.float32
                ),
                "bv": np.ascontiguousarray(
                    b_attn[2 * C + g * F : 2 * C + (g + 1) * F]
                ).astype(np.float32),
                # column-parallel c_proj: full rows, our 512 output columns
                "wp": np.ascontiguousarray(w_proj[:, g * F : (g + 1) * F]).astype(BF16),
                "bp": np.ascontiguousarray(b_proj[g * F : (g + 1) * F]).astype(
                    np.float32
                ),
            }
        )

    global _last_in_maps
    _last_in_maps = in_maps  # stashed for external profiling harnesses
    res = run_bass_kernel_spmd(nc, in_maps, core_ids=list(range(N_CORES)))

    # Each core's "out" is [T, 512]: all tokens, its 512 output columns.
    out = np.empty((B, T, C), dtype=np.float32)
    for b in range(B):
        out[b][:, 0:F] = res.results[2 * b]["out"]
        out[b][:, F:C] = res.results[2 * b + 1]["out"]
    return out


# revision 13
# speedup vs baseline: 1.1840x; 1.0338x over previous
"""Causal self-attention (B=4, T=2048, C=1024, NH=16) on 8 TRN2 NeuronCores.

Sharding: tensor-parallel over heads x data-parallel over batch.
Core i handles batch b = i//2 and head-group g = i%2 (8 heads each).
  - c_attn column-parallel: each core computes q,k,v for its 8 heads.
  - attention: fully local per core (its heads, its batch element).
  - c_proj COLUMN-parallel: after attention, the pair [2b, 2b+1] exchanges
    normalized head outputs y (bf16) via one small AllGather per
    (q-block, head-pair); each core then computes out[:, my 512 columns] for
    ALL tokens using its wp column slice (the rank-dependence lives in the
    host-provided wp/bp inputs, so the device program is rank-independent).
    No ReduceScatter, no fp32 partial traffic, no output copies: c_proj
    results go straight to the output tensor.

Device algorithm (per core), matmuls bf16 with fp32 PSUM accumulation:
  xT (C,T) staged transposed by host; inputs staged in dependency order so
  the first qkv matmuls start ~9us in.
  qT = wq^T @ xT, kT = wk^T @ xT   (feature-major, 512-token units)
  v  = x @ wv                      (token-major) + ones column per head
  per head pair (2fc, 2fc+1), per q-block Q (512 wide):
    s^T[kchunk] = kT_h^T @ qT_h    (K=64 contraction, row-tiled pair ->
        concurrent); columns trimmed to the causal range on diagonal chunks.
    p = exp(0.125 * s^T)  (ScalarE, bf16 out) over causal columns only; the
        128-wide diagonal triangle is zeroed by a DVE multiply with a
        precomputed triangular mask (GpSimd affine_select builds it once).
    o^T[65,W] += v_aug_h^T @ p     (ones column -> row 64 = softmax denom)
    r = approx_recip(denoms) on DVE; r is partition-broadcast on GpSimd
    (daisy chain, SBUF only -- no DMA round trip, no PSUM); y_h = o^T * bc,
    staged to DRAM and AllGathered across the pair. The broadcast +
    multiplies + exchange are deferred into the next head-pair's score
    stream so their waits never head-block a queue.
  c_proj pipelined one q-block behind, one 128-token block per head-pair
  slot, interleaved into the exp-bound attention phase as PE filler; its
  contraction orders the last-exchanged feature chunks last so it can start
  before the final AllGather lands.
"""

import sys

if "/opt/trn_rl_repo" not in sys.path:
    sys.path.insert(0, "/opt/trn_rl_repo")

import numpy as np
import ml_dtypes

import concourse.bass as bass
import concourse.bacc as bacc
import concourse.mybir as mybir
import concourse.tile as tile
from concourse.bass import ts, ds
from concourse.bass_utils import run_bass_kernel_spmd

BF16 = ml_dtypes.bfloat16
N_CORES = 8
B, T, C = 4, 2048, 1024
NH, HS = 16, 64
H_LOC = NH // 2        # heads per core
F = H_LOC * HS         # 512 local qkv features
NFC = F // 128         # 4 feature chunks (one head pair each)
NKC = T // 128         # 16 key chunks
NQ = T // 512          # 4 query blocks
KO = C // 128          # 8 contraction chunks for c_proj (full features)
REPLICA_GROUPS = [[0, 1], [2, 3], [4, 5], [6, 7]]
# c_proj contraction order: chunks from the last-exchanged head pairs last,
# so the token-block matmuls can start before the final AllGather lands
KO_ORDER = [0, 4, 1, 5, 2, 6, 3, 7]

FP32 = mybir.dt.float32
BF = mybir.dt.bfloat16


def _build_nc():
    # Bacc (not plain Bass): its compile() pipeline runs
    # generate_event_semaphores, which splits sync waits so no instruction
    # carries more than the hardware allows (walrus rejects >1 otherwise).
    nc = bacc.Bacc(None, target_bir_lowering=False, num_devices=N_CORES)

    xT = nc.dram_tensor("xT", [C, T], BF, kind="ExternalInput")
    wq = nc.dram_tensor("wq", [C, F], BF, kind="ExternalInput")
    wk = nc.dram_tensor("wk", [C, F], BF, kind="ExternalInput")
    wv = nc.dram_tensor("wv", [C, F], BF, kind="ExternalInput")
    bq = nc.dram_tensor("bq", [F], FP32, kind="ExternalInput")
    bk = nc.dram_tensor("bk", [F], FP32, kind="ExternalInput")
    bv = nc.dram_tensor("bv", [F], FP32, kind="ExternalInput")
    wp = nc.dram_tensor("wp", [C, F], BF, kind="ExternalInput")  # col slice
    bp = nc.dram_tensor("bp", [F], FP32, kind="ExternalInput")   # col slice
    out = nc.dram_tensor("out", [T, F], FP32, kind="ExternalOutput")

    with tile.TileContext(nc) as tc:
        _body(tc, xT, wq, wk, wv, bq, bk, bv, wp, bp, out)
    nc.compile()
    return nc


def _body(tc, xT, wq, wk, wv, bq, bk, bv, wp, bp, out):
    nc = tc.nc
    import contextlib

    ctx = contextlib.ExitStack()
    with ctx:
        wpool = ctx.enter_context(tc.tile_pool(name="weights", bufs=1))
        apool = ctx.enter_context(tc.tile_pool(name="acts", bufs=1))
        ppool = ctx.enter_context(tc.tile_pool(name="ptiles", bufs=3))
        npool = ctx.enter_context(tc.tile_pool(name="norm", bufs=2))
        yfpool = ctx.enter_context(tc.tile_pool(name="yfull", bufs=2))
        outp = ctx.enter_context(tc.tile_pool(name="outsb", bufs=3))
        # PSUM budget (8 banks): sAB [128,1024] x3 bufs = 6, oA/oB 1 each = 2
        ps_s = ctx.enter_context(tc.tile_pool(name="ps_s", bufs=3, space="PSUM"))
        ps_o = ctx.enter_context(tc.tile_pool(name="ps_o", bufs=1, space="PSUM"))
        dpool = ctx.enter_context(tc.tile_pool(name="dram", bufs=1, space="DRAM"))

        # ---- activation-table preload: a tiny exp up front so the ~2.7us
        # ACT_TABLE_LOAD overlaps input staging instead of the first score.
        warm_in = wpool.tile([1, 16], FP32)
        warm_out = wpool.tile([1, 16], BF)
        nc.vector.memset(warm_in, 0.0)
        nc.scalar.activation(
            out=warm_out, in_=warm_in,
            func=mybir.ActivationFunctionType.Exp, scale=1.0,
        )

        # ---- stage inputs into SBUF (ordered so compute starts early) ----
        wq_sb = wpool.tile([128, KO, F], BF)
        nc.sync.dma_start(out=wq_sb, in_=wq.rearrange("(ko p) f -> p ko f", p=128))
        bq_sb = wpool.tile([128, NFC], FP32)
        nc.sync.dma_start(out=bq_sb, in_=bq.rearrange("(fo p) -> p fo", p=128))
        # x staged in four token quarters so the first qk units start sooner
        x_sb = wpool.tile([128, KO, T], BF)
        x_re = xT.rearrange("(ko p) t -> p ko t", p=128)
        nc.sync.dma_start(out=x_sb[:, :, 0:512], in_=x_re[:, :, 0:512])
        wk_sb = wpool.tile([128, KO, F], BF)
        nc.sync.dma_start(out=wk_sb, in_=wk.rearrange("(ko p) f -> p ko f", p=128))
        bk_sb = wpool.tile([128, NFC], FP32)
        nc.sync.dma_start(out=bk_sb, in_=bk.rearrange("(fo p) -> p fo", p=128))
        wv_sb = wpool.tile([128, KO, F], BF)
        nc.sync.dma_start(out=wv_sb, in_=wv.rearrange("(ko p) f -> p ko f", p=128))
        # broadcast biases across partitions (for token-major layouts);
        # early: v_unit's bias add must not head-block the Vector queue
        bv_bc = wpool.tile([128, F], FP32)
        nc.sync.dma_start(
            out=bv_bc,
            in_=bass.AP(tensor=bv.ap().tensor, offset=0, ap=[[0, 128], [1, F]]),
        )
        nc.sync.dma_start(out=x_sb[:, :, 512:1024], in_=x_re[:, :, 512:1024])
        nc.sync.dma_start(out=x_sb[:, :, 1024:1536], in_=x_re[:, :, 1024:1536])
        nc.sync.dma_start(out=x_sb[:, :, 1536:2048], in_=x_re[:, :, 1536:2048])
        wp_sb = wpool.tile([128, KO, F], BF)
        nc.sync.dma_start(out=wp_sb, in_=wp.rearrange("(ko p) n -> p ko n", p=128))
        bp_bc = wpool.tile([128, F], FP32)
        nc.sync.dma_start(
            out=bp_bc,
            in_=bass.AP(tensor=bp.ap().tensor, offset=0, ap=[[0, 128], [1, F]]),
        )

        # ---- constants ----
        # triangular causal mask for the 128-wide diagonal band:
        # tri[p, h, t] = 1 if t >= p else 0  (query-offset t vs key p)
        tri = wpool.tile([128, 2, 128], BF)
        nc.vector.memset(tri, 1.0)
        nc.gpsimd.affine_select(
            out=tri, in_=tri,
            compare_op=mybir.AluOpType.is_ge,
            fill=0.0, base=0, channel_multiplier=-1,
            pattern=[[0, 2], [1, 128]],
        )

        # ---- persistent activations ----
        qT_sb = apool.tile([128, NFC, T], BF)   # q, feature-major
        kT_sb = apool.tile([128, NFC, T], BF)   # k, feature-major
        # v token-major, 66-stride per head: cols 0:64 = v, col 64 = ones
        v_sb = apool.tile([128, NKC, H_LOC, 66], BF)
        nc.vector.memset(v_sb[:, :, :, 64:65], 1.0)

        # per-(block, head-pair) AllGather staging. yd = our 128-feature
        # chunk of y^T for the block; ya[r] = rank r's chunk (rank 0 = head
        # group 0 = global feature chunk fc, rank 1 = chunk 4+fc).
        yd = [[dpool.tile([128, 512], BF, name=f"yd{q}_{f}") for f in range(NFC)]
              for q in range(NQ)]
        ya = [[dpool.tile([2, 128, 512], BF, name=f"ya{q}_{f}") for f in range(NFC)]
              for q in range(NQ)]
        yfs = {}  # Q -> gathered full-feature y^T [128, KO, 512] in SBUF

        # ---- qkv projection units (512-token granularity so they slot
        # finely into the attention phase as PE filler) ----
        def qk_unit(w_sb, b_sb, dst, fc, tq):
            ps = ps_s.tile([128, 1024], FP32, tag="sAB")
            for kc in range(KO):
                nc.tensor.matmul(
                    ps[:, 0:512],
                    lhsT=w_sb[:, kc, ts(fc, 128)],
                    rhs=x_sb[:, kc, ts(tq, 512)],
                    start=(kc == 0),
                    stop=(kc == KO - 1),
                )
            nc.scalar.activation(
                out=dst[:, fc, ts(tq, 512)],
                in_=ps[:, 0:512],
                func=mybir.ActivationFunctionType.Identity,
                bias=b_sb[:, fc : fc + 1],
                scale=1.0,
            )

        def v_unit(tc_i):
            ps = ps_s.tile([128, 1024], FP32, tag="sAB")
            for kc in range(KO):
                nc.tensor.matmul(
                    ps[:, 0:512],
                    lhsT=x_sb[:, kc, ts(tc_i, 128)],
                    rhs=wv_sb[:, kc, :],
                    start=(kc == 0),
                    stop=(kc == KO - 1),
                )
            nc.vector.tensor_add(
                out=v_sb[:, tc_i, :, 0:64],
                in0=ps[:, 0:512].rearrange("p (h f) -> p h f", h=H_LOC),
                in1=bv_bc.rearrange("p (h f) -> p h f", h=H_LOC),
            )

        # ---- c_proj: our 512 output columns for one 128-token block,
        # split so the chunks fed by the block's last AllGather (head pair
        # 3 -> global chunks 3 and 7) can be emitted separately ----
        def proj_tb_begin(Q, tb):
            yf = yfs[Q]
            ps = ps_s.tile([128, 1024], FP32, tag="sAB")
            for ko in KO_ORDER[:-2]:
                nc.tensor.matmul(
                    ps[:, 0:512],
                    lhsT=yf[:, ko, ts(tb, 128)],
                    rhs=wp_sb[:, ko, :],
                    start=(ko == KO_ORDER[0]),
                    stop=False,
                )
            return ps

        def proj_tb_end(Q, tb, ps):
            yf = yfs[Q]
            for ko in KO_ORDER[-2:]:
                nc.tensor.matmul(
                    ps[:, 0:512],
                    lhsT=yf[:, ko, ts(tb, 128)],
                    rhs=wp_sb[:, ko, :],
                    start=False,
                    stop=(ko == KO_ORDER[-1]),
                )
            o_sb = outp.tile([128, 512], FP32, tag="osb")
            nc.vector.tensor_add(out=o_sb, in0=ps[:, 0:512], in1=bp_bc)
            nc.sync.dma_start(
                out=out.ap()[ds(Q * 512 + tb * 128, 128), :], in_=o_sb
            )

        def proj_tb(Q, tb):
            proj_tb_end(Q, tb, proj_tb_begin(Q, tb))

        # ---- attention ----
        pending = []  # deferred normalization phase-2 closures

        def flush_pending():
            while pending:
                pending.pop(0)()

        def attention_block(Q, pre_fc=None, slot_fns=None):
            pre_fc = pre_fc or {}
            slot_fns = slot_fns or {}
            nkc = 4 * Q + 4  # causal: only key chunks 0 .. 4Q+3 contribute
            LAG = 2  # AV matmuls trail the QK/exp pipeline by this many chunks
            yf = yfpool.tile([128, KO, 512], BF, tag="yf")
            yfs[Q] = yf
            for fc in range(NFC):  # head pair (2fc, 2fc+1)
                for u in pre_fc.get(fc, ()):
                    u()
                to = ps_o.tile([128, 512], FP32, tag="oA")
                tb_ = ps_o.tile([128, 512], FP32, tag="oB")
                pbuf = {}

                def emit_av(kc, to=to, tb_=tb_, nkc=nkc, fc=fc):
                    pAB, q0 = pbuf.pop(kc)
                    w = 512 - q0
                    nc.tensor.matmul(
                        to[0:65, ds(q0, w)],
                        lhsT=v_sb[:, kc, 2 * fc, 0:65],
                        rhs=pAB[:, ds(q0, w)],
                        start=(kc == 0),
                        stop=(kc == nkc - 1),
                    )
                    nc.tensor.matmul(
                        tb_[0:65, ds(q0, w)],
                        lhsT=v_sb[:, kc, 2 * fc + 1, 0:65],
                        rhs=pAB[:, ds(512 + q0, w)],
                        start=(kc == 0),
                        stop=(kc == nkc - 1),
                    )

                for kc in range(nkc):
                    j = kc - 4 * Q  # >= 0 on the diagonal band
                    q0 = 128 * j if j > 0 else 0
                    w = 512 - q0
                    # heads A and B share one 2-bank psum tile: A in cols
                    # 0:512 (array rows 0:64), B in 512:1024 (rows 64:128);
                    # the row-tiled pair runs concurrently on the PE.
                    sAB = ps_s.tile([128, 1024], FP32, tag="sAB")
                    nc.tensor.matmul(
                        sAB[:, ds(q0, w)],
                        lhsT=kT_sb[0:64, fc, ts(kc, 128)],
                        rhs=qT_sb[0:64, fc, ds(Q * 512 + q0, w)],
                        start=True,
                        stop=True,
                        tile_position=(0, 0),
                    )
                    nc.tensor.matmul(
                        sAB[:, ds(512 + q0, w)],
                        lhsT=kT_sb[64:128, fc, ts(kc, 128)],
                        rhs=qT_sb[64:128, fc, ds(Q * 512 + q0, w)],
                        start=True,
                        stop=True,
                        tile_position=(64, 0),
                    )
                    if kc == 2:
                        flush_pending()
                    pAB = ppool.tile([128, 1024], BF, tag="pAB", bufs=4)
                    pABh = pAB.rearrange("p (h q) -> p h q", h=2)
                    sABh = sAB.rearrange("p (h q) -> p h q", h=2)
                    # exp only the causal columns (columns < q0 are never
                    # read downstream: the AV rhs is trimmed to match)
                    nc.scalar.activation(
                        out=pABh[:, :, ds(q0, w)],
                        in_=sABh[:, :, ds(q0, w)],
                        func=mybir.ActivationFunctionType.Exp,
                        scale=0.125,
                    )
                    if j >= 0:
                        # zero the 128-wide causal triangle (DVE multiply
                        # with the precomputed mask)
                        nc.vector.tensor_mul(
                            out=pABh[:, :, ds(q0, 128)],
                            in0=pABh[:, :, ds(q0, 128)],
                            in1=tri,
                        )
                    pbuf[kc] = (pAB, q0)
                    if kc >= LAG:
                        emit_av(kc - LAG)
                for kc in range(max(0, nkc - LAG), nkc):
                    emit_av(kc)

                # normalization phase 1 (DVE): copy o out of PSUM (freeing
                # the banks), stage the denominator rows to partition 0,
                # approx-reciprocal.
                oA_sb = npool.tile([65, 512], FP32, tag="oAsb")
                oB_sb = npool.tile([65, 512], FP32, tag="oBsb")
                nc.vector.tensor_copy(out=oA_sb, in_=to[0:65, :])
                nc.vector.tensor_copy(out=oB_sb, in_=tb_[0:65, :])
                rz = npool.tile([1, 1024], FP32, tag="rz")
                nc.vector.tensor_copy(out=rz[:, 0:512], in_=oA_sb[64:65, :])
                nc.vector.tensor_copy(out=rz[:, 512:1024], in_=oB_sb[64:65, :])
                r = npool.tile([1, 1024], FP32, tag="r")
                nc.vector.reciprocal_approx_fast(out=r, in_=rz)

                # phase 2 (GpSimd broadcast + DVE multiplies + exchange) is
                # deferred into the next head-pair's score stream so its
                # reciprocal wait never head-blocks the DVE/GpSimd queues.
                def phase2(Q=Q, fc=fc, oA_sb=oA_sb, oB_sb=oB_sb, r=r, yf=yf):
                    # broadcast r from partition 0 to 64 partitions via the
                    # GpSimd daisy chain -- no DMA round trip, no PSUM
                    bc = npool.tile([64, 1024], FP32, tag="bc")
                    nc.gpsimd.partition_broadcast(out_ap=bc, in_ap=r, channels=64)
                    ystA = npool.tile([64, 512], BF, tag="ystA")
                    ystB = npool.tile([64, 512], BF, tag="ystB")
                    nc.vector.tensor_mul(out=ystA, in0=oA_sb[0:64, :], in1=bc[:, 0:512])
                    nc.vector.tensor_mul(out=ystB, in0=oB_sb[0:64, :], in1=bc[:, 512:1024])
                    # stage our feature chunk to DRAM and exchange it
                    ydt = yd[Q][fc]
                    nc.sync.dma_start(out=ydt[ds(0, 64), :], in_=ystA)
                    nc.sync.dma_start(out=ydt[ds(64, 64), :], in_=ystB)
                    nc.gpsimd.collective_compute(
                        "AllGather",
                        mybir.AluOpType.bypass,
                        replica_groups=REPLICA_GROUPS,
                        ins=[ydt[:]],
                        outs=[ya[Q][fc][:]],
                    )
                    # reloads ride the GpSimd SWDGE queue: their AllGather
                    # wait must not head-block the Sync staging DMAs
                    nc.gpsimd.dma_start(out=yf[:, fc, :], in_=ya[Q][fc][0])
                    nc.gpsimd.dma_start(out=yf[:, 4 + fc, :], in_=ya[Q][fc][1])

                pending.append(phase2)

                for fn in slot_fns.get(fc, ()):
                    fn()

        # ---- software-pipelined schedule ----
        # prefix: exactly what attention(0) fc0 needs
        qk_unit(wq_sb, bq_sb, qT_sb, 0, 0)
        qk_unit(wk_sb, bk_sb, kT_sb, 0, 0)
        for i in range(4):
            v_unit(i)

        def qkq(fc, tq):
            return lambda: qk_unit(wq_sb, bq_sb, qT_sb, fc, tq)

        def qkk(fc, tq):
            return lambda: qk_unit(wk_sb, bk_sb, kT_sb, fc, tq)

        attention_block(
            0,
            pre_fc={f: [qkq(f, 0), qkk(f, 0)] for f in (1, 2, 3)},
            slot_fns={f: [qkq(f, 1), qkk(f, 1), (lambda i=f: v_unit(4 + i))]
                      for f in range(4)},
        )
        attention_block(
            1,
            slot_fns={
                f: [qkq(f, 2), qkk(f, 2), (lambda i=f: v_unit(8 + i)),
                    (lambda i=f: proj_tb(0, i))]
                for f in range(4)
            },
        )
        attention_block(
            2,
            slot_fns={
                f: [qkq(f, 3), qkk(f, 3), (lambda i=f: v_unit(12 + i)),
                    (lambda i=f: proj_tb(1, i))]
                for f in range(4)
            },
        )
        attention_block(
            3,
            slot_fns={f: [(lambda i=f: proj_tb(2, i))] for f in range(4)},
        )
        flush_pending()
        # tail: interleave the four token blocks' early chunks (served by
        # already-landed AllGathers) so they overlap the final exchange
        ps0 = proj_tb_begin(3, 0)
        ps1 = proj_tb_begin(3, 1)
        ps2 = proj_tb_begin(3, 2)
        proj_tb_end(3, 0, ps0)
        ps3 = proj_tb_begin(3, 3)
        proj_tb_end(3, 1, ps1)
        proj_tb_end(3, 2, ps2)
        proj_tb_end(3, 3, ps3)


_NC_CACHE = None


def _get_nc():
    global _NC_CACHE
    if _NC_CACHE is None:
        _NC_CACHE = _build_nc()
    return _NC_CACHE


def kernel(x, w_attn, b_attn, w_proj, b_proj):
    x = np.asarray(x)
    w_attn = np.asarray(w_attn)
    b_attn = np.asarray(b_attn)
    w_proj = np.asarray(w_proj)
    b_proj = np.asarray(b_proj)

    nc = _get_nc()

    in_maps = []
    for i in range(N_CORES):
        b, g = i // 2, i % 2
        in_maps.append(
            {
                "xT": np.ascontiguousarray(x[b].T).astype(BF16),
                "wq": np.ascontiguousarray(w_attn[:, g * F : (g + 1) * F]).astype(BF16),
                "wk": np.ascontiguousarray(
                    w_attn[:, C + g * F : C + (g + 1) * F]
                ).astype(BF16),
                "wv": np.ascontiguousarray(
                    w_attn[:, 2 * C + g * F : 2 * C + (g + 1) * F]
                ).astype(BF16),
                "bq": np.ascontiguousarray(b_attn[g * F : (g + 1) * F]).astype(
                    np.float32
                ),
                "bk": np.ascontiguousarray(b_attn[C + g * F : C + (g + 1) * F]).astype(
                    np.float32
                ),
                "bv": np.ascontiguousarray(
                    b_attn[2 * C + g * F : 2 * C + (g + 1) * F]
                ).astype(np.float32),
                # column-parallel c_proj: full rows, our 512 output columns
                "wp": np.ascontiguousarray(w_proj[:, g * F : (g + 1) * F]).astype(BF16),
                "bp": np.ascontiguousarray(b_proj[g * F : (g + 1) * F]).astype(
                    np.float32
                ),
            }
        )

    global _last_in_maps
    _last_in_maps = in_maps  # stashed for external profiling harnesses
    res = run_bass_kernel_spmd(nc, in_maps, core_ids=list(range(N_CORES)))

    # Each core's "out" is [T, 512]: all tokens, its 512 output columns.
    out = np.empty((B, T, C), dtype=np.float32)
    for b in range(B):
        out[b][:, 0:F] = res.results[2 * b]["out"]
        out[b][:, F:C] = res.results[2 * b + 1]["out"]
    return out


# revision 16
# speedup vs baseline: 1.2599x; 1.0642x over previous
"""Causal self-attention (B=4, T=2048, C=1024, NH=16) on 8 TRN2 NeuronCores.

Sharding: tensor-parallel over heads x data-parallel over batch.
Core i handles batch b = i//2 and head-group g = i%2 (8 heads each).
  - c_attn column-parallel: each core computes q,k,v for its 8 heads.
  - attention: fully local per core (its heads, its batch element).
  - c_proj COLUMN-parallel: after attention, the pair [2b, 2b+1] exchanges
    normalized head outputs y (bf16) via one small AllGather per
    (q-block, head-pair); each core then computes out[:, my 512 columns] for
    ALL tokens using its wp column slice (the rank-dependence lives in the
    host-provided wp/bp inputs, so the device program is rank-independent).
    No ReduceScatter, no fp32 partial traffic, no output copies: c_proj
    results go straight to the output tensor.

Device algorithm (per core), matmuls bf16 with fp32 PSUM accumulation:
  xT (C,T) staged transposed by host; inputs staged in dependency order so
  the first qkv matmuls start ~9us in.
  qT = wq^T @ xT, kT = wk^T @ xT   (feature-major, 512-token units)
  v  = x @ wv                      (token-major) + ones column per head
  per head pair (2fc, 2fc+1), per q-block Q (512 wide):
    s^T[kchunk] = kT_h^T @ qT_h    (K=64 contraction, row-tiled pair ->
        concurrent); columns trimmed to the causal range on diagonal chunks.
    p = exp(0.125 * s^T)  (ScalarE, bf16 out) over causal columns only; the
        128-wide diagonal triangle is zeroed by a DVE multiply with a
        precomputed triangular mask (GpSimd affine_select builds it once).
    o^T[65,W] += v_aug_h^T @ p     (ones column -> row 64 = softmax denom)
    r = approx_recip(denoms) on DVE; r is partition-broadcast on GpSimd
    (daisy chain, SBUF only -- no DMA round trip, no PSUM); y_h = o^T * bc,
    staged to DRAM and AllGathered across the pair. The broadcast +
    multiplies + exchange are deferred into the next head-pair's score
    stream so their waits never head-block a queue.
  c_proj pipelined one q-block behind, one 128-token block per head-pair
  slot, interleaved into the exp-bound attention phase as PE filler; its
  contraction orders the last-exchanged feature chunks last so it can start
  before the final AllGather lands.
"""

import sys

if "/opt/trn_rl_repo" not in sys.path:
    sys.path.insert(0, "/opt/trn_rl_repo")

import numpy as np
import ml_dtypes

import concourse.bass as bass
import concourse.bacc as bacc
import concourse.mybir as mybir
import concourse.tile as tile
from concourse.bass import ts, ds
from concourse.bass_utils import run_bass_kernel_spmd

BF16 = ml_dtypes.bfloat16
N_CORES = 8
B, T, C = 4, 2048, 1024
NH, HS = 16, 64
H_LOC = NH // 2        # heads per core
F = H_LOC * HS         # 512 local qkv features
NFC = F // 128         # 4 feature chunks (one head pair each)
NKC = T // 128         # 16 key chunks
NQ = T // 512          # 4 query blocks
KO = C // 128          # 8 contraction chunks for c_proj (full features)
REPLICA_GROUPS = [[0, 1], [2, 3], [4, 5], [6, 7]]
# c_proj contraction order: chunks from the last-exchanged head pairs last,
# so the token-block matmuls can start before the final AllGather lands
KO_ORDER = [0, 4, 1, 5, 2, 6, 3, 7]

FP32 = mybir.dt.float32
BF = mybir.dt.bfloat16


def _build_nc():
    # Bacc (not plain Bass): its compile() pipeline runs
    # generate_event_semaphores, which splits sync waits so no instruction
    # carries more than the hardware allows (walrus rejects >1 otherwise).
    nc = bacc.Bacc(None, target_bir_lowering=False, num_devices=N_CORES)

    xT = nc.dram_tensor("xT", [C, T], BF, kind="ExternalInput")
    wq = nc.dram_tensor("wq", [C, F], BF, kind="ExternalInput")
    wk = nc.dram_tensor("wk", [C, F], BF, kind="ExternalInput")
    wv = nc.dram_tensor("wv", [C, F], BF, kind="ExternalInput")
    bq = nc.dram_tensor("bq", [F], FP32, kind="ExternalInput")
    bk = nc.dram_tensor("bk", [F], FP32, kind="ExternalInput")
    bv = nc.dram_tensor("bv", [F], FP32, kind="ExternalInput")
    wp = nc.dram_tensor("wp", [C, F], BF, kind="ExternalInput")  # col slice
    bp = nc.dram_tensor("bp", [F], FP32, kind="ExternalInput")   # col slice
    out = nc.dram_tensor("out", [T, F], FP32, kind="ExternalOutput")

    with tile.TileContext(nc) as tc:
        _body(tc, xT, wq, wk, wv, bq, bk, bv, wp, bp, out)
    nc.compile()
    return nc


def _body(tc, xT, wq, wk, wv, bq, bk, bv, wp, bp, out):
    nc = tc.nc
    import contextlib

    ctx = contextlib.ExitStack()
    with ctx:
        wpool = ctx.enter_context(tc.tile_pool(name="weights", bufs=1))
        apool = ctx.enter_context(tc.tile_pool(name="acts", bufs=1))
        ppool = ctx.enter_context(tc.tile_pool(name="ptiles", bufs=3))
        npool = ctx.enter_context(tc.tile_pool(name="norm", bufs=2))
        yfpool = ctx.enter_context(tc.tile_pool(name="yfull", bufs=3))
        outp = ctx.enter_context(tc.tile_pool(name="outsb", bufs=3))
        # PSUM budget (8 banks): sAB [128,1024] x3 bufs = 6, oA/oB 1 each = 2
        ps_s = ctx.enter_context(tc.tile_pool(name="ps_s", bufs=3, space="PSUM"))
        ps_o = ctx.enter_context(tc.tile_pool(name="ps_o", bufs=1, space="PSUM"))
        dpool = ctx.enter_context(tc.tile_pool(name="dram", bufs=1, space="DRAM"))

        # ---- activation-table preload: a tiny exp up front so the ~2.7us
        # ACT_TABLE_LOAD overlaps input staging instead of the first score.
        warm_in = wpool.tile([1, 16], FP32)
        warm_out = wpool.tile([1, 16], BF)
        nc.vector.memset(warm_in, 0.0)
        nc.scalar.activation(
            out=warm_out, in_=warm_in,
            func=mybir.ActivationFunctionType.Exp, scale=1.0,
        )
        # ... and a tiny AllGather so the ~18us collective-stream bootstrap
        # overlaps staging instead of the first real exchange
        wg_in = dpool.tile([1, 16], BF, name="wg_in")
        wg_out = dpool.tile([2, 1, 16], BF, name="wg_out")
        nc.gpsimd.collective_compute(
            "AllGather",
            mybir.AluOpType.bypass,
            replica_groups=REPLICA_GROUPS,
            ins=[wg_in[:]],
            outs=[wg_out[:]],
        )

        # ---- stage inputs into SBUF (ordered so compute starts early) ----
        wq_sb = wpool.tile([128, KO, F], BF)
        nc.sync.dma_start(out=wq_sb, in_=wq.rearrange("(ko p) f -> p ko f", p=128))
        bq_sb = wpool.tile([128, NFC], FP32)
        nc.sync.dma_start(out=bq_sb, in_=bq.rearrange("(fo p) -> p fo", p=128))
        # x staged in four token quarters so the first qk units start sooner
        x_sb = wpool.tile([128, KO, T], BF)
        x_re = xT.rearrange("(ko p) t -> p ko t", p=128)
        nc.sync.dma_start(out=x_sb[:, :, 0:512], in_=x_re[:, :, 0:512])
        wk_sb = wpool.tile([128, KO, F], BF)
        nc.sync.dma_start(out=wk_sb, in_=wk.rearrange("(ko p) f -> p ko f", p=128))
        bk_sb = wpool.tile([128, NFC], FP32)
        nc.sync.dma_start(out=bk_sb, in_=bk.rearrange("(fo p) -> p fo", p=128))
        wv_sb = wpool.tile([128, KO, F], BF)
        nc.sync.dma_start(out=wv_sb, in_=wv.rearrange("(ko p) f -> p ko f", p=128))
        # broadcast biases across partitions (for token-major layouts);
        # early: v_unit's bias add must not head-block the Vector queue
        bv_bc = wpool.tile([128, F], FP32)
        nc.sync.dma_start(
            out=bv_bc,
            in_=bass.AP(tensor=bv.ap().tensor, offset=0, ap=[[0, 128], [1, F]]),
        )
        nc.sync.dma_start(out=x_sb[:, :, 512:1024], in_=x_re[:, :, 512:1024])
        nc.sync.dma_start(out=x_sb[:, :, 1024:1536], in_=x_re[:, :, 1024:1536])
        nc.sync.dma_start(out=x_sb[:, :, 1536:2048], in_=x_re[:, :, 1536:2048])
        wp_sb = wpool.tile([128, KO, F], BF)
        nc.sync.dma_start(out=wp_sb, in_=wp.rearrange("(ko p) n -> p ko n", p=128))
        bp_bc = wpool.tile([128, F], FP32)
        nc.sync.dma_start(
            out=bp_bc,
            in_=bass.AP(tensor=bp.ap().tensor, offset=0, ap=[[0, 128], [1, F]]),
        )

        # ---- constants ----
        # triangular causal mask for the 128-wide diagonal band:
        # tri[p, h, t] = 1 if t >= p else 0  (query-offset t vs key p)
        tri = wpool.tile([128, 2, 128], BF)
        nc.vector.memset(tri, 1.0)
        nc.gpsimd.affine_select(
            out=tri, in_=tri,
            compare_op=mybir.AluOpType.is_ge,
            fill=0.0, base=0, channel_multiplier=-1,
            pattern=[[0, 2], [1, 128]],
        )

        # ---- persistent activations ----
        qT_sb = apool.tile([128, NFC, T], BF)   # q, feature-major
        kT_sb = apool.tile([128, NFC, T], BF)   # k, feature-major
        # v token-major, 66-stride per head: cols 0:64 = v, col 64 = ones
        v_sb = apool.tile([128, NKC, H_LOC, 66], BF)
        nc.vector.memset(v_sb[:, :, :, 64:65], 1.0)

        # per-(block, head-pair) AllGather staging. yd = our 128-feature
        # chunk of y^T for the block; ya[r] = rank r's chunk (rank 0 = head
        # group 0 = global feature chunk fc, rank 1 = chunk 4+fc).
        yd = [[dpool.tile([128, 512], BF, name=f"yd{q}_{f}") for f in range(NFC)]
              for q in range(NQ)]
        ya = [[dpool.tile([2, 128, 512], BF, name=f"ya{q}_{f}") for f in range(NFC)]
              for q in range(NQ)]
        yfs = {}  # Q -> gathered full-feature y^T [128, KO, 512] in SBUF

        # ---- qkv projection units (512-token granularity so they slot
        # finely into the attention phase as PE filler) ----
        def qk_unit(w_sb, b_sb, dst, fc, tq):
            ps = ps_s.tile([128, 1024], FP32, tag="sAB")
            for kc in range(KO):
                nc.tensor.matmul(
                    ps[:, 0:512],
                    lhsT=w_sb[:, kc, ts(fc, 128)],
                    rhs=x_sb[:, kc, ts(tq, 512)],
                    start=(kc == 0),
                    stop=(kc == KO - 1),
                )
            nc.scalar.activation(
                out=dst[:, fc, ts(tq, 512)],
                in_=ps[:, 0:512],
                func=mybir.ActivationFunctionType.Identity,
                bias=b_sb[:, fc : fc + 1],
                scale=1.0,
            )

        def v_unit(tc_i):
            ps = ps_s.tile([128, 1024], FP32, tag="sAB")
            for kc in range(KO):
                nc.tensor.matmul(
                    ps[:, 0:512],
                    lhsT=x_sb[:, kc, ts(tc_i, 128)],
                    rhs=wv_sb[:, kc, :],
                    start=(kc == 0),
                    stop=(kc == KO - 1),
                )
            nc.vector.tensor_add(
                out=v_sb[:, tc_i, :, 0:64],
                in0=ps[:, 0:512].rearrange("p (h f) -> p h f", h=H_LOC),
                in1=bv_bc.rearrange("p (h f) -> p h f", h=H_LOC),
            )

        # ---- c_proj: our 512 output columns for one 128-token block,
        # split so the chunks fed by the block's last AllGather (head pair
        # 3 -> global chunks 3 and 7) can be emitted separately ----
        def proj_tb_begin(Q, tb):
            yf = yfs[Q]
            ps = ps_s.tile([128, 1024], FP32, tag="sAB")
            for ko in KO_ORDER[:-2]:
                nc.tensor.matmul(
                    ps[:, 0:512],
                    lhsT=yf[:, ko, ts(tb, 128)],
                    rhs=wp_sb[:, ko, :],
                    start=(ko == KO_ORDER[0]),
                    stop=False,
                )
            return ps

        def proj_tb_end(Q, tb, ps):
            yf = yfs[Q]
            for ko in KO_ORDER[-2:]:
                nc.tensor.matmul(
                    ps[:, 0:512],
                    lhsT=yf[:, ko, ts(tb, 128)],
                    rhs=wp_sb[:, ko, :],
                    start=False,
                    stop=(ko == KO_ORDER[-1]),
                )
            o_sb = outp.tile([128, 512], FP32, tag="osb")
            nc.vector.tensor_add(out=o_sb, in0=ps[:, 0:512], in1=bp_bc)
            nc.sync.dma_start(
                out=out.ap()[ds(Q * 512 + tb * 128, 128), :], in_=o_sb
            )

        def proj_tb(Q, tb):
            proj_tb_end(Q, tb, proj_tb_begin(Q, tb))

        # ---- attention ----
        pending = []  # deferred normalization phase-2 closures

        def flush_pending():
            while pending:
                pending.pop(0)()

        def attention_block(Q, pre_fc=None, slot_fns=None):
            pre_fc = pre_fc or {}
            slot_fns = slot_fns or {}
            nkc = 4 * Q + 4  # causal: only key chunks 0 .. 4Q+3 contribute
            LAG = 2  # AV matmuls trail the QK/exp pipeline by this many chunks
            yf = yfpool.tile([128, KO, 512], BF, tag="yf")
            yfs[Q] = yf
            for fc in range(NFC):  # head pair (2fc, 2fc+1)
                for u in pre_fc.get(fc, ()):
                    u()
                to = ps_o.tile([128, 512], FP32, tag="oA")
                tb_ = ps_o.tile([128, 512], FP32, tag="oB")
                pbuf = {}

                def emit_av(kc, to=to, tb_=tb_, nkc=nkc, fc=fc):
                    pAB, q0 = pbuf.pop(kc)
                    w = 512 - q0
                    nc.tensor.matmul(
                        to[0:65, ds(q0, w)],
                        lhsT=v_sb[:, kc, 2 * fc, 0:65],
                        rhs=pAB[:, ds(q0, w)],
                        start=(kc == 0),
                        stop=(kc == nkc - 1),
                    )
                    nc.tensor.matmul(
                        tb_[0:65, ds(q0, w)],
                        lhsT=v_sb[:, kc, 2 * fc + 1, 0:65],
                        rhs=pAB[:, ds(512 + q0, w)],
                        start=(kc == 0),
                        stop=(kc == nkc - 1),
                    )

                for kc in range(nkc):
                    j = kc - 4 * Q  # >= 0 on the diagonal band
                    q0 = 128 * j if j > 0 else 0
                    w = 512 - q0
                    # heads A and B share one 2-bank psum tile: A in cols
                    # 0:512 (array rows 0:64), B in 512:1024 (rows 64:128);
                    # the row-tiled pair runs concurrently on the PE.
                    sAB = ps_s.tile([128, 1024], FP32, tag="sAB")
                    nc.tensor.matmul(
                        sAB[:, ds(q0, w)],
                        lhsT=kT_sb[0:64, fc, ts(kc, 128)],
                        rhs=qT_sb[0:64, fc, ds(Q * 512 + q0, w)],
                        start=True,
                        stop=True,
                        tile_position=(0, 0),
                    )
                    nc.tensor.matmul(
                        sAB[:, ds(512 + q0, w)],
                        lhsT=kT_sb[64:128, fc, ts(kc, 128)],
                        rhs=qT_sb[64:128, fc, ds(Q * 512 + q0, w)],
                        start=True,
                        stop=True,
                        tile_position=(64, 0),
                    )
                    if kc == 2:
                        flush_pending()
                    pAB = ppool.tile([128, 1024], BF, tag="pAB", bufs=4)
                    pABh = pAB.rearrange("p (h q) -> p h q", h=2)
                    sABh = sAB.rearrange("p (h q) -> p h q", h=2)
                    # exp only the causal columns (columns < q0 are never
                    # read downstream: the AV rhs is trimmed to match)
                    nc.scalar.activation(
                        out=pABh[:, :, ds(q0, w)],
                        in_=sABh[:, :, ds(q0, w)],
                        func=mybir.ActivationFunctionType.Exp,
                        scale=0.125,
                    )
                    if j >= 0:
                        # zero the 128-wide causal triangle (DVE multiply
                        # with the precomputed mask)
                        nc.vector.tensor_mul(
                            out=pABh[:, :, ds(q0, 128)],
                            in0=pABh[:, :, ds(q0, 128)],
                            in1=tri,
                        )
                    pbuf[kc] = (pAB, q0)
                    if kc >= LAG:
                        emit_av(kc - LAG)
                for kc in range(max(0, nkc - LAG), nkc):
                    emit_av(kc)

                # normalization phase 1 (DVE): copy o out of PSUM (freeing
                # the banks), stage the denominator rows to partition 0,
                # approx-reciprocal.
                oA_sb = npool.tile([65, 512], FP32, tag="oAsb")
                oB_sb = npool.tile([65, 512], FP32, tag="oBsb")
                nc.vector.tensor_copy(out=oA_sb, in_=to[0:65, :])
                nc.vector.tensor_copy(out=oB_sb, in_=tb_[0:65, :])
                rz = npool.tile([1, 1024], FP32, tag="rz")
                nc.vector.tensor_copy(out=rz[:, 0:512], in_=oA_sb[64:65, :])
                nc.vector.tensor_copy(out=rz[:, 512:1024], in_=oB_sb[64:65, :])
                r = npool.tile([1, 1024], FP32, tag="r")
                nc.vector.reciprocal_approx_fast(out=r, in_=rz)

                # phase 2 (GpSimd broadcast + DVE multiplies + exchange) is
                # deferred into the next head-pair's score stream so its
                # reciprocal wait never head-blocks the DVE/GpSimd queues.
                def phase2(Q=Q, fc=fc, oA_sb=oA_sb, oB_sb=oB_sb, r=r, yf=yf):
                    # broadcast r from partition 0 to 64 partitions via the
                    # GpSimd daisy chain -- no DMA round trip, no PSUM
                    bc = npool.tile([64, 1024], FP32, tag="bc")
                    nc.gpsimd.partition_broadcast(out_ap=bc, in_ap=r, channels=64)
                    ystA = npool.tile([64, 512], BF, tag="ystA")
                    ystB = npool.tile([64, 512], BF, tag="ystB")
                    nc.vector.tensor_mul(out=ystA, in0=oA_sb[0:64, :], in1=bc[:, 0:512])
                    nc.vector.tensor_mul(out=ystB, in0=oB_sb[0:64, :], in1=bc[:, 512:1024])
                    # stage our feature chunk to DRAM and exchange it
                    ydt = yd[Q][fc]
                    nc.sync.dma_start(out=ydt[ds(0, 64), :], in_=ystA)
                    nc.sync.dma_start(out=ydt[ds(64, 64), :], in_=ystB)
                    nc.gpsimd.collective_compute(
                        "AllGather",
                        mybir.AluOpType.bypass,
                        replica_groups=REPLICA_GROUPS,
                        ins=[ydt[:]],
                        outs=[ya[Q][fc][:]],
                    )
                    # reloads ride the GpSimd SWDGE queue: their AllGather
                    # wait must not head-block the Sync staging DMAs
                    nc.gpsimd.dma_start(out=yf[:, fc, :], in_=ya[Q][fc][0])
                    nc.gpsimd.dma_start(out=yf[:, 4 + fc, :], in_=ya[Q][fc][1])

                pending.append(phase2)

                for fn in slot_fns.get(fc, ()):
                    fn()

        # ---- software-pipelined schedule ----
        # prefix: exactly what attention(0) fc0 needs
        qk_unit(wq_sb, bq_sb, qT_sb, 0, 0)
        qk_unit(wk_sb, bk_sb, kT_sb, 0, 0)
        for i in range(4):
            v_unit(i)

        def qkq(fc, tq):
            return lambda: qk_unit(wq_sb, bq_sb, qT_sb, fc, tq)

        def qkk(fc, tq):
            return lambda: qk_unit(wk_sb, bk_sb, kT_sb, fc, tq)

        attention_block(
            0,
            pre_fc={f: [qkq(f, 0), qkk(f, 0)] for f in (1, 2, 3)},
            slot_fns={f: [qkq(f, 1), qkk(f, 1), (lambda i=f: v_unit(4 + i))]
                      for f in range(4)},
        )
        attention_block(
            1,
            slot_fns={
                f: [qkq(f, 2), qkk(f, 2), (lambda i=f: v_unit(8 + i)),
                    (lambda i=f: proj_tb(0, i))]
                for f in range(4)
            },
        )
        attention_block(
            2,
            slot_fns={
                f: [qkq(f, 3), qkk(f, 3), (lambda i=f: v_unit(12 + i))]
                for f in range(4)
            },
        )
        # block 3 is otherwise exp-bound with an idle (HAM-cooling) PE --
        # both pending c_proj blocks ride there as filler
        attention_block(
            3,
            slot_fns={
                0: [lambda: proj_tb(1, 0), lambda: proj_tb(1, 1)],
                1: [lambda: proj_tb(1, 2), lambda: proj_tb(1, 3)],
                2: [lambda: proj_tb(2, 0), lambda: proj_tb(2, 1)],
                3: [lambda: proj_tb(2, 2), lambda: proj_tb(2, 3)],
            },
        )
        flush_pending()
        # tail: interleave the four token blocks' early chunks (served by
        # already-landed AllGathers) so they overlap the final exchange
        ps0 = proj_tb_begin(3, 0)
        ps1 = proj_tb_begin(3, 1)
        ps2 = proj_tb_begin(3, 2)
        proj_tb_end(3, 0, ps0)
        ps3 = proj_tb_begin(3, 3)
        proj_tb_end(3, 1, ps1)
        proj_tb_end(3, 2, ps2)
        proj_tb_end(3, 3, ps3)


_NC_CACHE = None


def _get_nc():
    global _NC_CACHE
    if _NC_CACHE is None:
        _NC_CACHE = _build_nc()
    return _NC_CACHE


def kernel(x, w_attn, b_attn, w_proj, b_proj):
    x = np.asarray(x)
    w_attn = np.asarray(w_attn)
    b_attn = np.asarray(b_attn)
    w_proj = np.asarray(w_proj)
    b_proj = np.asarray(b_proj)

    nc = _get_nc()

    in_maps = []
    for i in range(N_CORES):
        b, g = i // 2, i % 2
        in_maps.append(
            {
                "xT": np.ascontiguousarray(x[b].T).astype(BF16),
                "wq": np.ascontiguousarray(w_attn[:, g * F : (g + 1) * F]).astype(BF16),
                "wk": np.ascontiguousarray(
                    w_attn[:, C + g * F : C + (g + 1) * F]
                ).astype(BF16),
                "wv": np.ascontiguousarray(
                    w_attn[:, 2 * C + g * F : 2 * C + (g + 1) * F]
                ).astype(BF16),
                "bq": np.ascontiguousarray(b_attn[g * F : (g + 1) * F]).astype(
                    np.float32
                ),
                "bk": np.ascontiguousarray(b_attn[C + g * F : C + (g + 1) * F]).astype(
                    np.float32
                ),
                "bv": np.ascontiguousarray(
                    b_attn[2 * C + g * F : 2 * C + (g + 1) * F]
                ).astype(np.float32),
                # column-parallel c_proj: full rows, our 512 output columns
                "wp": np.ascontiguousarray(w_proj[:, g * F : (g + 1) * F]).astype(BF16),
                "bp": np.ascontiguousarray(b_proj[g * F : (g + 1) * F]).astype(
                    np.float32
                ),
            }
        )

    global _last_in_maps
    _last_in_maps = in_maps  # stashed for external profiling harnesses
    res = run_bass_kernel_spmd(nc, in_maps, core_ids=list(range(N_CORES)))

    # Each core's "out" is [T, 512]: all tokens, its 512 output columns.
    out = np.empty((B, T, C), dtype=np.float32)
    for b in range(B):
        out[b][:, 0:F] = res.results[2 * b]["out"]
        out[b][:, F:C] = res.results[2 * b + 1]["out"]
    return out


# revision 20
# speedup vs baseline: 1.2776x; 1.0140x over previous
"""Causal self-attention (B=4, T=2048, C=1024, NH=16) on 8 TRN2 NeuronCores.

Sharding: tensor-parallel over heads x data-parallel over batch.
Core i handles batch b = i//2 and head-group g = i%2 (8 heads each).
  - c_attn column-parallel: each core computes q,k,v for its 8 heads.
  - attention: fully local per core (its heads, its batch element).
  - c_proj COLUMN-parallel: after attention, the pair [2b, 2b+1] exchanges
    normalized head outputs y (bf16) via one small AllGather per
    (q-block, head-pair); each core then computes out[:, my 512 columns] for
    ALL tokens using its wp column slice (the rank-dependence lives in the
    host-provided wp/bp inputs, so the device program is rank-independent).
    No ReduceScatter, no fp32 partial traffic, no output copies: c_proj
    results go straight to the output tensor.

Device algorithm (per core), matmuls bf16 with fp32 PSUM accumulation:
  xT (C,T) staged transposed by host; inputs staged in dependency order so
  the first qkv matmuls start ~9us in.
  qT = wq^T @ xT, kT = wk^T @ xT   (feature-major, 512-token units)
  v  = x @ wv                      (token-major) + ones column per head
  per head pair (2fc, 2fc+1), per q-block Q (512 wide):
    s^T[kchunk] = kT_h^T @ qT_h    (K=64 contraction, row-tiled pair ->
        concurrent); columns trimmed to the causal range on diagonal chunks.
    p = exp(0.125 * s^T)  (ScalarE, bf16 out) over causal columns only; the
        128-wide diagonal triangle is zeroed by a DVE multiply with a
        precomputed triangular mask (GpSimd affine_select builds it once).
    o^T[65,W] += v_aug_h^T @ p     (ones column -> row 64 = softmax denom)
    r = approx_recip(denoms) on DVE; r is partition-broadcast on GpSimd
    (daisy chain, SBUF only -- no DMA round trip, no PSUM); y_h = o^T * bc,
    staged to DRAM and AllGathered across the pair. The broadcast +
    multiplies + exchange are deferred into the next head-pair's score
    stream so their waits never head-block a queue.
  c_proj pipelined one q-block behind, one 128-token block per head-pair
  slot, interleaved into the exp-bound attention phase as PE filler; its
  contraction orders the last-exchanged feature chunks last so it can start
  before the final AllGather lands.
"""

import sys

if "/opt/trn_rl_repo" not in sys.path:
    sys.path.insert(0, "/opt/trn_rl_repo")

import numpy as np
import ml_dtypes

import concourse.bass as bass
import concourse.bacc as bacc
import concourse.mybir as mybir
import concourse.tile as tile
from concourse.bass import ts, ds
from concourse.bass_utils import run_bass_kernel_spmd

BF16 = ml_dtypes.bfloat16
N_CORES = 8
B, T, C = 4, 2048, 1024
NH, HS = 16, 64
H_LOC = NH // 2        # heads per core
F = H_LOC * HS         # 512 local qkv features
NFC = F // 128         # 4 feature chunks (one head pair each)
NKC = T // 128         # 16 key chunks
NQ = T // 512          # 4 query blocks
KO = C // 128          # 8 contraction chunks for c_proj (full features)
REPLICA_GROUPS = [[0, 1], [2, 3], [4, 5], [6, 7]]
# c_proj contraction order: chunks from the last-exchanged head pairs last,
# so the token-block matmuls can start before the final AllGather lands
KO_ORDER = [0, 4, 1, 5, 2, 6, 3, 7]

FP32 = mybir.dt.float32
BF = mybir.dt.bfloat16


def _build_nc():
    # Bacc (not plain Bass): its compile() pipeline runs
    # generate_event_semaphores, which splits sync waits so no instruction
    # carries more than the hardware allows (walrus rejects >1 otherwise).
    nc = bacc.Bacc(None, target_bir_lowering=False, num_devices=N_CORES)

    xT = nc.dram_tensor("xT", [C, T], BF, kind="ExternalInput")
    wq = nc.dram_tensor("wq", [C, F], BF, kind="ExternalInput")
    wk = nc.dram_tensor("wk", [C, F], BF, kind="ExternalInput")
    wv = nc.dram_tensor("wv", [C, F], BF, kind="ExternalInput")
    bq = nc.dram_tensor("bq", [F], FP32, kind="ExternalInput")
    bk = nc.dram_tensor("bk", [F], FP32, kind="ExternalInput")
    bv = nc.dram_tensor("bv", [F], FP32, kind="ExternalInput")
    wp = nc.dram_tensor("wp", [C, F], BF, kind="ExternalInput")  # col slice
    bp = nc.dram_tensor("bp", [F], FP32, kind="ExternalInput")   # col slice
    out = nc.dram_tensor("out", [T, F], FP32, kind="ExternalOutput")

    with tile.TileContext(nc) as tc:
        _body(tc, xT, wq, wk, wv, bq, bk, bv, wp, bp, out)
    nc.compile()
    return nc


def _body(tc, xT, wq, wk, wv, bq, bk, bv, wp, bp, out):
    nc = tc.nc
    import contextlib

    ctx = contextlib.ExitStack()
    with ctx:
        wpool = ctx.enter_context(tc.tile_pool(name="weights", bufs=1))
        apool = ctx.enter_context(tc.tile_pool(name="acts", bufs=1))
        ppool = ctx.enter_context(tc.tile_pool(name="ptiles", bufs=3))
        npool = ctx.enter_context(tc.tile_pool(name="norm", bufs=2))
        yfpool = ctx.enter_context(tc.tile_pool(name="yfull", bufs=3))
        outp = ctx.enter_context(tc.tile_pool(name="outsb", bufs=3))
        # PSUM budget (8 banks): sAB [128,1024] x3 bufs = 6, oA/oB 1 each = 2
        ps_s = ctx.enter_context(tc.tile_pool(name="ps_s", bufs=3, space="PSUM"))
        ps_o = ctx.enter_context(tc.tile_pool(name="ps_o", bufs=1, space="PSUM"))
        dpool = ctx.enter_context(tc.tile_pool(name="dram", bufs=1, space="DRAM"))

        # ---- activation-table preload: a tiny exp up front so the ~2.7us
        # ACT_TABLE_LOAD overlaps input staging instead of the first score.
        warm_in = wpool.tile([1, 16], FP32)
        warm_out = wpool.tile([1, 16], BF)
        nc.vector.memset(warm_in, 0.0)
        nc.scalar.activation(
            out=warm_out, in_=warm_in,
            func=mybir.ActivationFunctionType.Exp, scale=1.0,
        )
        # ... and a tiny AllGather so the ~18us collective-stream bootstrap
        # overlaps staging instead of the first real exchange
        wg_in = dpool.tile([1, 16], BF, name="wg_in")
        wg_out = dpool.tile([2, 1, 16], BF, name="wg_out")
        nc.gpsimd.collective_compute(
            "AllGather",
            mybir.AluOpType.bypass,
            replica_groups=REPLICA_GROUPS,
            ins=[wg_in[:]],
            outs=[wg_out[:]],
        )

        # ---- stage inputs into SBUF (ordered so compute starts early) ----
        wq_sb = wpool.tile([128, KO, F], BF)
        nc.sync.dma_start(out=wq_sb, in_=wq.rearrange("(ko p) f -> p ko f", p=128))
        bq_sb = wpool.tile([128, NFC], FP32)
        nc.sync.dma_start(out=bq_sb, in_=bq.rearrange("(fo p) -> p fo", p=128))
        # x staged in four token quarters so the first qk units start sooner
        x_sb = wpool.tile([128, KO, T], BF)
        x_re = xT.rearrange("(ko p) t -> p ko t", p=128)
        nc.sync.dma_start(out=x_sb[:, :, 0:512], in_=x_re[:, :, 0:512])
        wk_sb = wpool.tile([128, KO, F], BF)
        nc.sync.dma_start(out=wk_sb, in_=wk.rearrange("(ko p) f -> p ko f", p=128))
        bk_sb = wpool.tile([128, NFC], FP32)
        nc.sync.dma_start(out=bk_sb, in_=bk.rearrange("(fo p) -> p fo", p=128))
        wv_sb = wpool.tile([128, KO, F], BF)
        nc.sync.dma_start(out=wv_sb, in_=wv.rearrange("(ko p) f -> p ko f", p=128))
        # broadcast biases across partitions (for token-major layouts);
        # early: v_unit's bias add must not head-block the Vector queue
        bv_bc = wpool.tile([128, F], FP32)
        nc.sync.dma_start(
            out=bv_bc,
            in_=bass.AP(tensor=bv.ap().tensor, offset=0, ap=[[0, 128], [1, F]]),
        )
        nc.sync.dma_start(out=x_sb[:, :, 512:1024], in_=x_re[:, :, 512:1024])
        nc.sync.dma_start(out=x_sb[:, :, 1024:1536], in_=x_re[:, :, 1024:1536])
        nc.sync.dma_start(out=x_sb[:, :, 1536:2048], in_=x_re[:, :, 1536:2048])
        wp_sb = wpool.tile([128, KO, F], BF)
        nc.sync.dma_start(out=wp_sb, in_=wp.rearrange("(ko p) n -> p ko n", p=128))
        bp_bc = wpool.tile([128, F], FP32)
        nc.sync.dma_start(
            out=bp_bc,
            in_=bass.AP(tensor=bp.ap().tensor, offset=0, ap=[[0, 128], [1, F]]),
        )

        # ---- constants ----
        # triangular causal mask for the 128-wide diagonal band:
        # tri[p, h, t] = 1 if t >= p else 0  (query-offset t vs key p)
        tri = wpool.tile([128, 2, 128], BF)
        nc.vector.memset(tri, 1.0)
        nc.gpsimd.affine_select(
            out=tri, in_=tri,
            compare_op=mybir.AluOpType.is_ge,
            fill=0.0, base=0, channel_multiplier=-1,
            pattern=[[0, 2], [1, 128]],
        )

        # ---- persistent activations ----
        qT_sb = apool.tile([128, NFC, T], BF)   # q, feature-major
        kT_sb = apool.tile([128, NFC, T], BF)   # k, feature-major
        # v token-major, 66-stride per head: cols 0:64 = v, col 64 = ones
        v_sb = apool.tile([128, NKC, H_LOC, 66], BF)
        nc.vector.memset(v_sb[:, :, :, 64:65], 1.0)

        # per-(block, head-pair) AllGather staging. yd = our 128-feature
        # chunk of y^T for the block; ya[r] = rank r's chunk (rank 0 = head
        # group 0 = global feature chunk fc, rank 1 = chunk 4+fc).
        yd = [[dpool.tile([128, 512], BF, name=f"yd{q}_{f}") for f in range(NFC)]
              for q in range(NQ)]
        ya = [[dpool.tile([2, 128, 512], BF, name=f"ya{q}_{f}") for f in range(NFC)]
              for q in range(NQ)]
        yfs = {}  # Q -> gathered full-feature y^T [128, KO, 512] in SBUF

        # ---- qkv projection units (512-token granularity so they slot
        # finely into the attention phase as PE filler) ----
        def qk_unit(w_sb, b_sb, dst, fc, tq):
            ps = ps_s.tile([128, 1024], FP32, tag="sAB")
            for kc in range(KO):
                nc.tensor.matmul(
                    ps[:, 0:512],
                    lhsT=w_sb[:, kc, ts(fc, 128)],
                    rhs=x_sb[:, kc, ts(tq, 512)],
                    start=(kc == 0),
                    stop=(kc == KO - 1),
                )
            nc.scalar.activation(
                out=dst[:, fc, ts(tq, 512)],
                in_=ps[:, 0:512],
                func=mybir.ActivationFunctionType.Identity,
                bias=b_sb[:, fc : fc + 1],
                scale=1.0,
            )

        def v_unit(tc_i):
            ps = ps_s.tile([128, 1024], FP32, tag="sAB")
            for kc in range(KO):
                nc.tensor.matmul(
                    ps[:, 0:512],
                    lhsT=x_sb[:, kc, ts(tc_i, 128)],
                    rhs=wv_sb[:, kc, :],
                    start=(kc == 0),
                    stop=(kc == KO - 1),
                )
            nc.vector.tensor_add(
                out=v_sb[:, tc_i, :, 0:64],
                in0=ps[:, 0:512].rearrange("p (h f) -> p h f", h=H_LOC),
                in1=bv_bc.rearrange("p (h f) -> p h f", h=H_LOC),
            )

        # ---- c_proj: our 512 output columns for one 128-token block,
        # split so the chunks fed by the block's last AllGather (head pair
        # 3 -> global chunks 3 and 7) can be emitted separately ----
        def proj_tb_begin(Q, tb):
            yf = yfs[Q]
            ps = ps_s.tile([128, 1024], FP32, tag="sAB")
            for ko in KO_ORDER[:-2]:
                nc.tensor.matmul(
                    ps[:, 0:512],
                    lhsT=yf[:, ko, ts(tb, 128)],
                    rhs=wp_sb[:, ko, :],
                    start=(ko == KO_ORDER[0]),
                    stop=False,
                )
            return ps

        def proj_tb_end(Q, tb, ps):
            yf = yfs[Q]
            for ko in KO_ORDER[-2:]:
                nc.tensor.matmul(
                    ps[:, 0:512],
                    lhsT=yf[:, ko, ts(tb, 128)],
                    rhs=wp_sb[:, ko, :],
                    start=False,
                    stop=(ko == KO_ORDER[-1]),
                )
            o_sb = outp.tile([128, 512], FP32, tag="osb")
            nc.vector.tensor_add(out=o_sb, in0=ps[:, 0:512], in1=bp_bc)
            nc.sync.dma_start(
                out=out.ap()[ds(Q * 512 + tb * 128, 128), :], in_=o_sb
            )

        def proj_tb(Q, tb):
            proj_tb_end(Q, tb, proj_tb_begin(Q, tb))

        # ---- attention ----
        pending = []  # deferred normalization phase-2 closures

        def flush_pending():
            while pending:
                pending.pop(0)()

        def attention_block(Q, pre_fc=None, slot_fns=None):
            pre_fc = pre_fc or {}
            slot_fns = slot_fns or {}
            nkc = 4 * Q + 4  # causal: only key chunks 0 .. 4Q+3 contribute
            LAG = 2  # AV matmuls trail the QK/exp pipeline by this many chunks
            yf = yfpool.tile([128, KO, 512], BF, tag="yf")
            yfs[Q] = yf
            for fc in range(NFC):  # head pair (2fc, 2fc+1)
                for u in pre_fc.get(fc, ()):
                    u()
                # spread this head-pair's filler work (qkv units / c_proj
                # blocks) evenly through the chunk loop: a ~2us unit is
                # absorbed by the 2-chunk exp backlog, while a single big
                # burst at the boundary starves the exp pipeline
                fns = list(slot_fns.get(fc, ()))
                spots = {}
                for i_f in range(len(fns)):
                    pos = min(nkc - 1, (i_f + 1) * nkc // (len(fns) + 1))
                    spots.setdefault(pos, []).append(fns[i_f])
                to = ps_o.tile([128, 512], FP32, tag="oA")
                tb_ = ps_o.tile([128, 512], FP32, tag="oB")
                pbuf = {}

                def emit_av(kc, to=to, tb_=tb_, nkc=nkc, fc=fc):
                    pAB, q0 = pbuf.pop(kc)
                    w = 512 - q0
                    nc.tensor.matmul(
                        to[0:65, ds(q0, w)],
                        lhsT=v_sb[:, kc, 2 * fc, 0:65],
                        rhs=pAB[:, ds(q0, w)],
                        start=(kc == 0),
                        stop=(kc == nkc - 1),
                    )
                    nc.tensor.matmul(
                        tb_[0:65, ds(q0, w)],
                        lhsT=v_sb[:, kc, 2 * fc + 1, 0:65],
                        rhs=pAB[:, ds(512 + q0, w)],
                        start=(kc == 0),
                        stop=(kc == nkc - 1),
                    )

                for kc in range(nkc):
                    j = kc - 4 * Q  # >= 0 on the diagonal band
                    q0 = 128 * j if j > 0 else 0
                    w = 512 - q0
                    # heads A and B share one 2-bank psum tile: A in cols
                    # 0:512 (array rows 0:64), B in 512:1024 (rows 64:128);
                    # the row-tiled pair runs concurrently on the PE.
                    sAB = ps_s.tile([128, 1024], FP32, tag="sAB")
                    nc.tensor.matmul(
                        sAB[:, ds(q0, w)],
                        lhsT=kT_sb[0:64, fc, ts(kc, 128)],
                        rhs=qT_sb[0:64, fc, ds(Q * 512 + q0, w)],
                        start=True,
                        stop=True,
                        tile_position=(0, 0),
                    )
                    nc.tensor.matmul(
                        sAB[:, ds(512 + q0, w)],
                        lhsT=kT_sb[64:128, fc, ts(kc, 128)],
                        rhs=qT_sb[64:128, fc, ds(Q * 512 + q0, w)],
                        start=True,
                        stop=True,
                        tile_position=(64, 0),
                    )
                    if kc == 2:
                        flush_pending()
                    pAB = ppool.tile([128, 1024], BF, tag="pAB", bufs=4)
                    pABh = pAB.rearrange("p (h q) -> p h q", h=2)
                    sABh = sAB.rearrange("p (h q) -> p h q", h=2)
                    # exp only the causal columns (columns < q0 are never
                    # read downstream: the AV rhs is trimmed to match)
                    nc.scalar.activation(
                        out=pABh[:, :, ds(q0, w)],
                        in_=sABh[:, :, ds(q0, w)],
                        func=mybir.ActivationFunctionType.Exp,
                        scale=0.125,
                    )
                    if j >= 0:
                        # zero the 128-wide causal triangle (DVE multiply
                        # with the precomputed mask)
                        nc.vector.tensor_mul(
                            out=pABh[:, :, ds(q0, 128)],
                            in0=pABh[:, :, ds(q0, 128)],
                            in1=tri,
                        )
                    pbuf[kc] = (pAB, q0)
                    if kc >= LAG:
                        emit_av(kc - LAG)
                    for fn in spots.get(kc, ()):
                        fn()
                for kc in range(max(0, nkc - LAG), nkc):
                    emit_av(kc)

                # normalization phase 1 (DVE): copy o out of PSUM (freeing
                # the banks), stage the denominator rows to partition 0,
                # approx-reciprocal.
                oA_sb = npool.tile([65, 512], FP32, tag="oAsb")
                oB_sb = npool.tile([65, 512], FP32, tag="oBsb")
                nc.vector.tensor_copy(out=oA_sb, in_=to[0:65, :])
                nc.vector.tensor_copy(out=oB_sb, in_=tb_[0:65, :])
                rz = npool.tile([1, 1024], FP32, tag="rz")
                nc.vector.tensor_copy(out=rz[:, 0:512], in_=oA_sb[64:65, :])
                nc.vector.tensor_copy(out=rz[:, 512:1024], in_=oB_sb[64:65, :])
                r = npool.tile([1, 1024], FP32, tag="r")
                nc.vector.reciprocal_approx_fast(out=r, in_=rz)

                # phase 2 (GpSimd broadcast + DVE multiplies + exchange) is
                # deferred into the next head-pair's score stream so its
                # reciprocal wait never head-blocks the DVE/GpSimd queues.
                def phase2(Q=Q, fc=fc, oA_sb=oA_sb, oB_sb=oB_sb, r=r, yf=yf):
                    # broadcast r from partition 0 to 64 partitions via the
                    # GpSimd daisy chain -- no DMA round trip, no PSUM
                    bc = npool.tile([64, 1024], FP32, tag="bc")
                    nc.gpsimd.partition_broadcast(out_ap=bc, in_ap=r, channels=64)
                    ystA = npool.tile([64, 512], BF, tag="ystA")
                    ystB = npool.tile([64, 512], BF, tag="ystB")
                    nc.vector.tensor_mul(out=ystA, in0=oA_sb[0:64, :], in1=bc[:, 0:512])
                    nc.vector.tensor_mul(out=ystB, in0=oB_sb[0:64, :], in1=bc[:, 512:1024])
                    # stage our feature chunk to DRAM and exchange it
                    ydt = yd[Q][fc]
                    nc.sync.dma_start(out=ydt[ds(0, 64), :], in_=ystA)
                    nc.sync.dma_start(out=ydt[ds(64, 64), :], in_=ystB)
                    nc.gpsimd.collective_compute(
                        "AllGather",
                        mybir.AluOpType.bypass,
                        replica_groups=REPLICA_GROUPS,
                        ins=[ydt[:]],
                        outs=[ya[Q][fc][:]],
                    )
                    # reloads ride the GpSimd SWDGE queue: their AllGather
                    # wait must not head-block the Sync staging DMAs. In the
                    # last block they go to Sync instead, so the final
                    # AllGather's doorbell isn't queued behind them.
                    rq = nc.sync if Q == NQ - 1 else nc.gpsimd
                    rq.dma_start(out=yf[:, fc, :], in_=ya[Q][fc][0])
                    rq.dma_start(out=yf[:, 4 + fc, :], in_=ya[Q][fc][1])

                pending.append(phase2)

        # ---- software-pipelined schedule ----
        # prefix: exactly what attention(0) fc0 needs
        qk_unit(wq_sb, bq_sb, qT_sb, 0, 0)
        qk_unit(wk_sb, bk_sb, kT_sb, 0, 0)
        for i in range(4):
            v_unit(i)

        def qkq(fc, tq):
            return lambda: qk_unit(wq_sb, bq_sb, qT_sb, fc, tq)

        def qkk(fc, tq):
            return lambda: qk_unit(wk_sb, bk_sb, kT_sb, fc, tq)

        attention_block(
            0,
            pre_fc={f: [qkq(f, 0), qkk(f, 0)] for f in (1, 2, 3)},
            slot_fns={f: [qkq(f, 1), qkk(f, 1), (lambda i=f: v_unit(4 + i))]
                      for f in range(4)},
        )
        attention_block(
            1,
            slot_fns={
                f: [qkq(f, 2), qkk(f, 2), (lambda i=f: v_unit(8 + i)),
                    (lambda i=f: proj_tb(0, i))]
                for f in range(4)
            },
        )
        attention_block(
            2,
            slot_fns={
                f: [qkq(f, 3), qkk(f, 3), (lambda i=f: v_unit(12 + i))]
                for f in range(4)
            },
        )
        # block 3 is otherwise exp-bound with an idle (HAM-cooling) PE --
        # both pending c_proj blocks ride there as filler
        attention_block(
            3,
            slot_fns={
                0: [lambda: proj_tb(1, 0), lambda: proj_tb(1, 1)],
                1: [lambda: proj_tb(1, 2), lambda: proj_tb(1, 3)],
                2: [lambda: proj_tb(2, 0), lambda: proj_tb(2, 1)],
                3: [lambda: proj_tb(2, 2), lambda: proj_tb(2, 3)],
            },
        )
        flush_pending()
        # tail: interleave the four token blocks' early chunks (served by
        # already-landed AllGathers) so they overlap the final exchange
        ps0 = proj_tb_begin(3, 0)
        ps1 = proj_tb_begin(3, 1)
        ps2 = proj_tb_begin(3, 2)
        proj_tb_end(3, 0, ps0)
        ps3 = proj_tb_begin(3, 3)
        proj_tb_end(3, 1, ps1)
        proj_tb_end(3, 2, ps2)
        proj_tb_end(3, 3, ps3)


_NC_CACHE = None


def _get_nc():
    global _NC_CACHE
    if _NC_CACHE is None:
        _NC_CACHE = _build_nc()
    return _NC_CACHE


def kernel(x, w_attn, b_attn, w_proj, b_proj):
    x = np.asarray(x)
    w_attn = np.asarray(w_attn)
    b_attn = np.asarray(b_attn)
    w_proj = np.asarray(w_proj)
    b_proj = np.asarray(b_proj)

    nc = _get_nc()

    in_maps = []
    for i in range(N_CORES):
        b, g = i // 2, i % 2
        in_maps.append(
            {
                "xT": np.ascontiguousarray(x[b].T).astype(BF16),
                "wq": np.ascontiguousarray(w_attn[:, g * F : (g + 1) * F]).astype(BF16),
                "wk": np.ascontiguousarray(
                    w_attn[:, C + g * F : C + (g + 1) * F]
                ).astype(BF16),
                "wv": np.ascontiguousarray(
                    w_attn[:, 2 * C + g * F : 2 * C + (g + 1) * F]
                ).astype(BF16),
                "bq": np.ascontiguousarray(b_attn[g * F : (g + 1) * F]).astype(
                    np.float32
                ),
                "bk": np.ascontiguousarray(b_attn[C + g * F : C + (g + 1) * F]).astype(
                    np.float32
                ),
                "bv": np.ascontiguousarray(
                    b_attn[2 * C + g * F : 2 * C + (g + 1) * F]
                ).astype(np.float32),
                # column-parallel c_proj: full rows, our 512 output columns
                "wp": np.ascontiguousarray(w_proj[:, g * F : (g + 1) * F]).astype(BF16),
                "bp": np.ascontiguousarray(b_proj[g * F : (g + 1) * F]).astype(
                    np.float32
                ),
            }
        )

    global _last_in_maps
    _last_in_maps = in_maps  # stashed for external profiling harnesses
    res = run_bass_kernel_spmd(nc, in_maps, core_ids=list(range(N_CORES)))

    # Each core's "out" is [T, 512]: all tokens, its 512 output columns.
    out = np.empty((B, T, C), dtype=np.float32)
    for b in range(B):
        out[b][:, 0:F] = res.results[2 * b]["out"]
        out[b][:, F:C] = res.results[2 * b + 1]["out"]
    return out


# revision 23
# speedup vs baseline: 1.2897x; 1.0095x over previous
"""Causal self-attention (B=4, T=2048, C=1024, NH=16) on 8 TRN2 NeuronCores.

Sharding: tensor-parallel over heads x data-parallel over batch.
Core i handles batch b = i//2 and head-group g = i%2 (8 heads each).
  - c_attn column-parallel: each core computes q,k,v for its 8 heads.
  - attention: fully local per core (its heads, its batch element).
  - c_proj COLUMN-parallel: after attention, the pair [2b, 2b+1] exchanges
    normalized head outputs y (bf16) via one small AllGather per
    (q-block, head-pair); each core then computes out[:, my 512 columns] for
    ALL tokens using its wp column slice (the rank-dependence lives in the
    host-provided wp/bp inputs, so the device program is rank-independent).
    No ReduceScatter, no fp32 partial traffic, no output copies: c_proj
    results go straight to the output tensor.

Device algorithm (per core), matmuls bf16 with fp32 PSUM accumulation:
  xT (C,T) staged transposed by host; inputs staged in dependency order so
  the first qkv matmuls start ~9us in.
  qT = wq^T @ xT, kT = wk^T @ xT   (feature-major, 512-token units)
  v  = x @ wv                      (token-major) + ones column per head
  per head pair (2fc, 2fc+1), per q-block Q (512 wide):
    s^T[kchunk] = kT_h^T @ qT_h    (K=64 contraction, row-tiled pair ->
        concurrent); columns trimmed to the causal range on diagonal chunks.
    p = exp(0.125 * s^T)  (ScalarE, bf16 out) over causal columns only; the
        128-wide diagonal triangle is zeroed by a DVE multiply with a
        precomputed triangular mask (GpSimd affine_select builds it once).
    o^T[65,W] += v_aug_h^T @ p     (ones column -> row 64 = softmax denom)
    r = approx_recip(denoms) on DVE; r is partition-broadcast on GpSimd
    (daisy chain, SBUF only -- no DMA round trip, no PSUM); y_h = o^T * bc,
    staged to DRAM and AllGathered across the pair. The broadcast +
    multiplies + exchange are deferred into the next head-pair's score
    stream so their waits never head-block a queue.
  c_proj pipelined one q-block behind, one 128-token block per head-pair
  slot, interleaved into the exp-bound attention phase as PE filler; its
  contraction orders the last-exchanged feature chunks last so it can start
  before the final AllGather lands.
"""

import sys

if "/opt/trn_rl_repo" not in sys.path:
    sys.path.insert(0, "/opt/trn_rl_repo")

import numpy as np
import ml_dtypes

import concourse.bass as bass
import concourse.bacc as bacc
import concourse.mybir as mybir
import concourse.tile as tile
from concourse.bass import ts, ds
from concourse.bass_utils import run_bass_kernel_spmd

BF16 = ml_dtypes.bfloat16
N_CORES = 8
B, T, C = 4, 2048, 1024
NH, HS = 16, 64
H_LOC = NH // 2        # heads per core
F = H_LOC * HS         # 512 local qkv features
NFC = F // 128         # 4 feature chunks (one head pair each)
NKC = T // 128         # 16 key chunks
NQ = T // 512          # 4 query blocks
KO = C // 128          # 8 contraction chunks for c_proj (full features)
REPLICA_GROUPS = [[0, 1], [2, 3], [4, 5], [6, 7]]
# c_proj contraction order: chunks from the last-exchanged head pairs last,
# so the token-block matmuls can start before the final AllGather lands
KO_ORDER = [0, 4, 1, 5, 2, 6, 3, 7]

FP32 = mybir.dt.float32
BF = mybir.dt.bfloat16


def _build_nc():
    # Bacc (not plain Bass): its compile() pipeline runs
    # generate_event_semaphores, which splits sync waits so no instruction
    # carries more than the hardware allows (walrus rejects >1 otherwise).
    nc = bacc.Bacc(None, target_bir_lowering=False, num_devices=N_CORES)

    xT = nc.dram_tensor("xT", [C, T], BF, kind="ExternalInput")
    wq = nc.dram_tensor("wq", [C, F], BF, kind="ExternalInput")
    wk = nc.dram_tensor("wk", [C, F], BF, kind="ExternalInput")
    wv = nc.dram_tensor("wv", [C, F], BF, kind="ExternalInput")
    bq = nc.dram_tensor("bq", [F], FP32, kind="ExternalInput")
    bk = nc.dram_tensor("bk", [F], FP32, kind="ExternalInput")
    bv = nc.dram_tensor("bv", [F], FP32, kind="ExternalInput")
    wp = nc.dram_tensor("wp", [C, F], BF, kind="ExternalInput")  # col slice
    bp = nc.dram_tensor("bp", [F], FP32, kind="ExternalInput")   # col slice
    out = nc.dram_tensor("out", [T, F], FP32, kind="ExternalOutput")

    with tile.TileContext(nc) as tc:
        _body(tc, xT, wq, wk, wv, bq, bk, bv, wp, bp, out)
    nc.compile()
    return nc


def _body(tc, xT, wq, wk, wv, bq, bk, bv, wp, bp, out):
    nc = tc.nc
    import contextlib

    ctx = contextlib.ExitStack()
    with ctx:
        wpool = ctx.enter_context(tc.tile_pool(name="weights", bufs=1))
        apool = ctx.enter_context(tc.tile_pool(name="acts", bufs=1))
        ppool = ctx.enter_context(tc.tile_pool(name="ptiles", bufs=3))
        npool = ctx.enter_context(tc.tile_pool(name="norm", bufs=2))
        yfpool = ctx.enter_context(tc.tile_pool(name="yfull", bufs=3))
        outp = ctx.enter_context(tc.tile_pool(name="outsb", bufs=3))
        # PSUM budget (8 banks): sAB [128,1024] x3 bufs = 6, oA/oB 1 each = 2
        ps_s = ctx.enter_context(tc.tile_pool(name="ps_s", bufs=3, space="PSUM"))
        ps_o = ctx.enter_context(tc.tile_pool(name="ps_o", bufs=1, space="PSUM"))
        dpool = ctx.enter_context(tc.tile_pool(name="dram", bufs=1, space="DRAM"))

        # ---- activation-table preload: a tiny exp up front so the ~2.7us
        # ACT_TABLE_LOAD overlaps input staging instead of the first score.
        warm_in = wpool.tile([1, 16], FP32)
        warm_out = wpool.tile([1, 16], BF)
        nc.vector.memset(warm_in, 0.0)
        nc.scalar.activation(
            out=warm_out, in_=warm_in,
            func=mybir.ActivationFunctionType.Exp, scale=1.0,
        )
        # ... and a tiny AllGather so the ~18us collective-stream bootstrap
        # overlaps staging instead of the first real exchange
        wg_in = dpool.tile([1, 16], BF, name="wg_in")
        wg_out = dpool.tile([2, 1, 16], BF, name="wg_out")
        nc.gpsimd.collective_compute(
            "AllGather",
            mybir.AluOpType.bypass,
            replica_groups=REPLICA_GROUPS,
            ins=[wg_in[:]],
            outs=[wg_out[:]],
        )

        # ---- stage inputs into SBUF (ordered so compute starts early) ----
        wq_sb = wpool.tile([128, KO, F], BF)
        nc.sync.dma_start(out=wq_sb, in_=wq.rearrange("(ko p) f -> p ko f", p=128))
        bq_sb = wpool.tile([128, NFC], FP32)
        nc.sync.dma_start(out=bq_sb, in_=bq.rearrange("(fo p) -> p fo", p=128))
        # x staged in four token quarters so the first qk units start sooner
        x_sb = wpool.tile([128, KO, T], BF)
        x_re = xT.rearrange("(ko p) t -> p ko t", p=128)
        nc.sync.dma_start(out=x_sb[:, :, 0:512], in_=x_re[:, :, 0:512])
        wk_sb = wpool.tile([128, KO, F], BF)
        nc.sync.dma_start(out=wk_sb, in_=wk.rearrange("(ko p) f -> p ko f", p=128))
        bk_sb = wpool.tile([128, NFC], FP32)
        nc.sync.dma_start(out=bk_sb, in_=bk.rearrange("(fo p) -> p fo", p=128))
        wv_sb = wpool.tile([128, KO, F], BF)
        nc.sync.dma_start(out=wv_sb, in_=wv.rearrange("(ko p) f -> p ko f", p=128))
        # broadcast biases across partitions (for token-major layouts);
        # early: v_unit's bias add must not head-block the Vector queue
        bv_bc = wpool.tile([128, F], FP32)
        nc.sync.dma_start(
            out=bv_bc,
            in_=bass.AP(tensor=bv.ap().tensor, offset=0, ap=[[0, 128], [1, F]]),
        )
        nc.sync.dma_start(out=x_sb[:, :, 512:1024], in_=x_re[:, :, 512:1024])
        nc.sync.dma_start(out=x_sb[:, :, 1024:1536], in_=x_re[:, :, 1024:1536])
        nc.sync.dma_start(out=x_sb[:, :, 1536:2048], in_=x_re[:, :, 1536:2048])
        wp_sb = wpool.tile([128, KO, F], BF)
        nc.sync.dma_start(out=wp_sb, in_=wp.rearrange("(ko p) n -> p ko n", p=128))
        bp_bc = wpool.tile([128, F], FP32)
        nc.sync.dma_start(
            out=bp_bc,
            in_=bass.AP(tensor=bp.ap().tensor, offset=0, ap=[[0, 128], [1, F]]),
        )

        # ---- constants ----
        # triangular causal mask for the 128-wide diagonal band:
        # tri[p, h, t] = 1 if t >= p else 0  (query-offset t vs key p)
        tri = wpool.tile([128, 2, 128], BF)
        nc.vector.memset(tri, 1.0)
        nc.gpsimd.affine_select(
            out=tri, in_=tri,
            compare_op=mybir.AluOpType.is_ge,
            fill=0.0, base=0, channel_multiplier=-1,
            pattern=[[0, 2], [1, 128]],
        )

        # ---- persistent activations ----
        qT_sb = apool.tile([128, NFC, T], BF)   # q, feature-major
        kT_sb = apool.tile([128, NFC, T], BF)   # k, feature-major
        # v token-major, 66-stride per head: cols 0:64 = v, col 64 = ones
        v_sb = apool.tile([128, NKC, H_LOC, 66], BF)
        nc.vector.memset(v_sb[:, :, :, 64:65], 1.0)

        # per-(block, head-pair) AllGather staging. yd = our 128-feature
        # chunk of y^T for the block; ya[r] = rank r's chunk (rank 0 = head
        # group 0 = global feature chunk fc, rank 1 = chunk 4+fc).
        yd = [[dpool.tile([128, 512], BF, name=f"yd{q}_{f}") for f in range(NFC)]
              for q in range(NQ)]
        ya = [[dpool.tile([2, 128, 512], BF, name=f"ya{q}_{f}") for f in range(NFC)]
              for q in range(NQ)]
        yfs = {}  # Q -> gathered full-feature y^T [128, KO, 512] in SBUF

        # ---- qkv projection units (512-token granularity so they slot
        # finely into the attention phase as PE filler) ----
        def qk_unit(w_sb, b_sb, dst, fc, tq):
            ps = ps_s.tile([128, 1024], FP32, tag="sAB")
            for kc in range(KO):
                nc.tensor.matmul(
                    ps[:, 0:512],
                    lhsT=w_sb[:, kc, ts(fc, 128)],
                    rhs=x_sb[:, kc, ts(tq, 512)],
                    start=(kc == 0),
                    stop=(kc == KO - 1),
                )
            nc.scalar.activation(
                out=dst[:, fc, ts(tq, 512)],
                in_=ps[:, 0:512],
                func=mybir.ActivationFunctionType.Identity,
                bias=b_sb[:, fc : fc + 1],
                scale=1.0,
            )

        def v_unit(tc_i):
            ps = ps_s.tile([128, 1024], FP32, tag="sAB")
            for kc in range(KO):
                nc.tensor.matmul(
                    ps[:, 0:512],
                    lhsT=x_sb[:, kc, ts(tc_i, 128)],
                    rhs=wv_sb[:, kc, :],
                    start=(kc == 0),
                    stop=(kc == KO - 1),
                )
            nc.vector.tensor_add(
                out=v_sb[:, tc_i, :, 0:64],
                in0=ps[:, 0:512].rearrange("p (h f) -> p h f", h=H_LOC),
                in1=bv_bc.rearrange("p (h f) -> p h f", h=H_LOC),
            )

        # ---- c_proj: our 512 output columns for one 128-token block,
        # split so the chunks fed by the block's last AllGather (head pair
        # 3 -> global chunks 3 and 7) can be emitted separately ----
        def proj_tb_begin(Q, tb):
            yf = yfs[Q]
            ps = ps_s.tile([128, 1024], FP32, tag="sAB")
            for ko in KO_ORDER[:-2]:
                nc.tensor.matmul(
                    ps[:, 0:512],
                    lhsT=yf[:, ko, ts(tb, 128)],
                    rhs=wp_sb[:, ko, :],
                    start=(ko == KO_ORDER[0]),
                    stop=False,
                )
            return ps

        def proj_tb_end(Q, tb, ps):
            yf = yfs[Q]
            for ko in KO_ORDER[-2:]:
                nc.tensor.matmul(
                    ps[:, 0:512],
                    lhsT=yf[:, ko, ts(tb, 128)],
                    rhs=wp_sb[:, ko, :],
                    start=False,
                    stop=(ko == KO_ORDER[-1]),
                )
            o_sb = outp.tile([128, 512], FP32, tag="osb")
            nc.vector.tensor_add(out=o_sb, in0=ps[:, 0:512], in1=bp_bc)
            nc.sync.dma_start(
                out=out.ap()[ds(Q * 512 + tb * 128, 128), :], in_=o_sb
            )

        def proj_tb(Q, tb):
            proj_tb_end(Q, tb, proj_tb_begin(Q, tb))

        # ---- attention ----
        pending = []  # deferred normalization phase-2 closures
        pending_reload = []  # last-block reloads, deferred one more flush so
        # their AllGather wait is already over when they hit the queue

        def flush_pending():
            # old reloads first: their AllGathers are a full flush old, so
            # they dequeue without waiting and never head-block a doorbell
            while pending_reload:
                pending_reload.pop(0)()
            while pending:
                pending.pop(0)()

        def attention_block(Q, pre_fc=None, slot_fns=None):
            pre_fc = pre_fc or {}
            slot_fns = slot_fns or {}
            nkc = 4 * Q + 4  # causal: only key chunks 0 .. 4Q+3 contribute
            LAG = 2  # AV matmuls trail the QK/exp pipeline by this many chunks
            yf = yfpool.tile([128, KO, 512], BF, tag="yf")
            yfs[Q] = yf
            for fc in range(NFC):  # head pair (2fc, 2fc+1)
                for u in pre_fc.get(fc, ()):
                    u()
                # spread this head-pair's filler work (qkv units / c_proj
                # blocks) evenly through the chunk loop: a ~2us unit is
                # absorbed by the 2-chunk exp backlog, while a single big
                # burst at the boundary starves the exp pipeline
                fns = list(slot_fns.get(fc, ()))
                spots = {}
                for i_f in range(len(fns)):
                    pos = min(nkc - 1, (i_f + 1) * nkc // (len(fns) + 1))
                    spots.setdefault(pos, []).append(fns[i_f])
                to = ps_o.tile([128, 512], FP32, tag="oA")
                tb_ = ps_o.tile([128, 512], FP32, tag="oB")
                pbuf = {}

                def emit_av(kc, to=to, tb_=tb_, nkc=nkc, fc=fc):
                    pAB, q0 = pbuf.pop(kc)
                    w = 512 - q0
                    nc.tensor.matmul(
                        to[0:65, ds(q0, w)],
                        lhsT=v_sb[:, kc, 2 * fc, 0:65],
                        rhs=pAB[:, ds(q0, w)],
                        start=(kc == 0),
                        stop=(kc == nkc - 1),
                    )
                    nc.tensor.matmul(
                        tb_[0:65, ds(q0, w)],
                        lhsT=v_sb[:, kc, 2 * fc + 1, 0:65],
                        rhs=pAB[:, ds(512 + q0, w)],
                        start=(kc == 0),
                        stop=(kc == nkc - 1),
                    )

                for kc in range(nkc):
                    j = kc - 4 * Q  # >= 0 on the diagonal band
                    q0 = 128 * j if j > 0 else 0
                    w = 512 - q0
                    # heads A and B share one 2-bank psum tile: A in cols
                    # 0:512 (array rows 0:64), B in 512:1024 (rows 64:128);
                    # the row-tiled pair runs concurrently on the PE.
                    sAB = ps_s.tile([128, 1024], FP32, tag="sAB")
                    nc.tensor.matmul(
                        sAB[:, ds(q0, w)],
                        lhsT=kT_sb[0:64, fc, ts(kc, 128)],
                        rhs=qT_sb[0:64, fc, ds(Q * 512 + q0, w)],
                        start=True,
                        stop=True,
                        tile_position=(0, 0),
                    )
                    nc.tensor.matmul(
                        sAB[:, ds(512 + q0, w)],
                        lhsT=kT_sb[64:128, fc, ts(kc, 128)],
                        rhs=qT_sb[64:128, fc, ds(Q * 512 + q0, w)],
                        start=True,
                        stop=True,
                        tile_position=(64, 0),
                    )
                    if kc == 2:
                        flush_pending()
                    pAB = ppool.tile([128, 1024], BF, tag="pAB", bufs=4)
                    pABh = pAB.rearrange("p (h q) -> p h q", h=2)
                    sABh = sAB.rearrange("p (h q) -> p h q", h=2)
                    # exp only the causal columns (columns < q0 are never
                    # read downstream: the AV rhs is trimmed to match)
                    nc.scalar.activation(
                        out=pABh[:, :, ds(q0, w)],
                        in_=sABh[:, :, ds(q0, w)],
                        func=mybir.ActivationFunctionType.Exp,
                        scale=0.125,
                    )
                    if j >= 0:
                        # zero the 128-wide causal triangle (DVE multiply
                        # with the precomputed mask)
                        nc.vector.tensor_mul(
                            out=pABh[:, :, ds(q0, 128)],
                            in0=pABh[:, :, ds(q0, 128)],
                            in1=tri,
                        )
                    pbuf[kc] = (pAB, q0)
                    if kc >= LAG:
                        emit_av(kc - LAG)
                    for fn in spots.get(kc, ()):
                        fn()
                for kc in range(max(0, nkc - LAG), nkc):
                    emit_av(kc)

                # normalization phase 1 (DVE): copy o out of PSUM (freeing
                # the banks), stage the denominator rows to partition 0,
                # approx-reciprocal.
                oA_sb = npool.tile([65, 512], FP32, tag="oAsb")
                oB_sb = npool.tile([65, 512], FP32, tag="oBsb")
                nc.vector.tensor_copy(out=oA_sb, in_=to[0:65, :])
                nc.vector.tensor_copy(out=oB_sb, in_=tb_[0:65, :])
                rz = npool.tile([1, 1024], FP32, tag="rz")
                nc.vector.tensor_copy(out=rz[:, 0:512], in_=oA_sb[64:65, :])
                nc.vector.tensor_copy(out=rz[:, 512:1024], in_=oB_sb[64:65, :])
                r = npool.tile([1, 1024], FP32, tag="r")
                nc.vector.reciprocal_approx_fast(out=r, in_=rz)

                # phase 2 (GpSimd broadcast + DVE multiplies + exchange) is
                # deferred into the next head-pair's score stream so its
                # reciprocal wait never head-blocks the DVE/GpSimd queues.
                def phase2(Q=Q, fc=fc, oA_sb=oA_sb, oB_sb=oB_sb, r=r, yf=yf):
                    # broadcast r from partition 0 to 64 partitions via the
                    # GpSimd daisy chain -- no DMA round trip, no PSUM
                    bc = npool.tile([64, 1024], FP32, tag="bc")
                    nc.gpsimd.partition_broadcast(out_ap=bc, in_ap=r, channels=64)
                    ystA = npool.tile([64, 512], BF, tag="ystA")
                    ystB = npool.tile([64, 512], BF, tag="ystB")
                    nc.vector.tensor_mul(out=ystA, in0=oA_sb[0:64, :], in1=bc[:, 0:512])
                    nc.vector.tensor_mul(out=ystB, in0=oB_sb[0:64, :], in1=bc[:, 512:1024])
                    # stage our feature chunk to DRAM and exchange it
                    ydt = yd[Q][fc]
                    nc.sync.dma_start(out=ydt[ds(0, 64), :], in_=ystA)
                    nc.sync.dma_start(out=ydt[ds(64, 64), :], in_=ystB)
                    nc.gpsimd.collective_compute(
                        "AllGather",
                        mybir.AluOpType.bypass,
                        replica_groups=REPLICA_GROUPS,
                        ins=[ydt[:]],
                        outs=[ya[Q][fc][:]],
                    )
                    # reloads ride the GpSimd SWDGE queue: their AllGather
                    # wait must not head-block the Sync staging DMAs. In the
                    # last block they are deferred one further flush so they
                    # also never delay the next AllGather's doorbell.
                    def reload(Q=Q, fc=fc, yf=yf):
                        nc.gpsimd.dma_start(out=yf[:, fc, :], in_=ya[Q][fc][0])
                        nc.gpsimd.dma_start(out=yf[:, 4 + fc, :], in_=ya[Q][fc][1])

                    if Q == NQ - 1:
                        pending_reload.append(reload)
                    else:
                        reload()

                pending.append(phase2)

        # ---- software-pipelined schedule ----
        # prefix: exactly what attention(0) fc0 needs
        qk_unit(wq_sb, bq_sb, qT_sb, 0, 0)
        qk_unit(wk_sb, bk_sb, kT_sb, 0, 0)
        for i in range(4):
            v_unit(i)

        def qkq(fc, tq):
            return lambda: qk_unit(wq_sb, bq_sb, qT_sb, fc, tq)

        def qkk(fc, tq):
            return lambda: qk_unit(wk_sb, bk_sb, kT_sb, fc, tq)

        attention_block(
            0,
            pre_fc={f: [qkq(f, 0), qkk(f, 0)] for f in (1, 2, 3)},
            slot_fns={f: [qkq(f, 1), qkk(f, 1), (lambda i=f: v_unit(4 + i))]
                      for f in range(4)},
        )
        attention_block(
            1,
            slot_fns={
                f: [qkq(f, 2), qkk(f, 2), (lambda i=f: v_unit(8 + i)),
                    (lambda i=f: proj_tb(0, i))]
                for f in range(4)
            },
        )
        attention_block(
            2,
            slot_fns={
                f: [qkq(f, 3), qkk(f, 3), (lambda i=f: v_unit(12 + i))]
                for f in range(4)
            },
        )
        # block 3 is otherwise exp-bound with an idle (HAM-cooling) PE --
        # both pending c_proj blocks ride there as filler
        attention_block(
            3,
            slot_fns={
                0: [lambda: proj_tb(1, 0), lambda: proj_tb(1, 1)],
                1: [lambda: proj_tb(1, 2), lambda: proj_tb(1, 3)],
                2: [lambda: proj_tb(2, 0), lambda: proj_tb(2, 1)],
                3: [lambda: proj_tb(2, 2), lambda: proj_tb(2, 3)],
            },
        )
        flush_pending()
        # tail: interleave the four token blocks' early chunks (served by
        # already-landed AllGathers) so they overlap the final exchange
        ps0 = proj_tb_begin(3, 0)
        ps1 = proj_tb_begin(3, 1)
        ps2 = proj_tb_begin(3, 2)
        while pending_reload:  # the final exchange's reload
            pending_reload.pop(0)()
        proj_tb_end(3, 0, ps0)
        ps3 = proj_tb_begin(3, 3)
        proj_tb_end(3, 1, ps1)
        proj_tb_end(3, 2, ps2)
        proj_tb_end(3, 3, ps3)


_NC_CACHE = None


def _get_nc():
    global _NC_CACHE
    if _NC_CACHE is None:
        _NC_CACHE = _build_nc()
    return _NC_CACHE


def kernel(x, w_attn, b_attn, w_proj, b_proj):
    x = np.asarray(x)
    w_attn = np.asarray(w_attn)
    b_attn = np.asarray(b_attn)
    w_proj = np.asarray(w_proj)
    b_proj = np.asarray(b_proj)

    nc = _get_nc()

    in_maps = []
    for i in range(N_CORES):
        b, g = i // 2, i % 2
        in_maps.append(
            {
                "xT": np.ascontiguousarray(x[b].T).astype(BF16),
                "wq": np.ascontiguousarray(w_attn[:, g * F : (g + 1) * F]).astype(BF16),
                "wk": np.ascontiguousarray(
                    w_attn[:, C + g * F : C + (g + 1) * F]
                ).astype(BF16),
                "wv": np.ascontiguousarray(
                    w_attn[:, 2 * C + g * F : 2 * C + (g + 1) * F]
                ).astype(BF16),
                "bq": np.ascontiguousarray(b_attn[g * F : (g + 1) * F]).astype(
                    np.float32
                ),
                "bk": np.ascontiguousarray(b_attn[C + g * F : C + (g + 1) * F]).astype(
                    np.float32
                ),
                "bv": np.ascontiguousarray(
                    b_attn[2 * C + g * F : 2 * C + (g + 1) * F]
                ).astype(np.float32),
                # column-parallel c_proj: full rows, our 512 output columns
                "wp": np.ascontiguousarray(w_proj[:, g * F : (g + 1) * F]).astype(BF16),
                "bp": np.ascontiguousarray(b_proj[g * F : (g + 1) * F]).astype(
                    np.float32
                ),
            }
        )

    global _last_in_maps
    _last_in_maps = in_maps  # stashed for external profiling harnesses
    res = run_bass_kernel_spmd(nc, in_maps, core_ids=list(range(N_CORES)))

    # Each core's "out" is [T, 512]: all tokens, its 512 output columns.
    out = np.empty((B, T, C), dtype=np.float32)
    for b in range(B):
        out[b][:, 0:F] = res.results[2 * b]["out"]
        out[b][:, F:C] = res.results[2 * b + 1]["out"]
    return out


# revision 28
# speedup vs baseline: 1.3050x; 1.0119x over previous
"""Causal self-attention (B=4, T=2048, C=1024, NH=16) on 8 TRN2 NeuronCores.

Sharding: tensor-parallel over heads x data-parallel over batch.
Core i handles batch b = i//2 and head-group g = i%2 (8 heads each).
  - c_attn column-parallel: each core computes q,k,v for its 8 heads.
  - attention: fully local per core (its heads, its batch element).
  - c_proj COLUMN-parallel: after attention, the pair [2b, 2b+1] exchanges
    normalized head outputs y (bf16) via one small AllGather per
    (q-block, head-pair); each core then computes out[:, my 512 columns] for
    ALL tokens using its wp column slice (the rank-dependence lives in the
    host-provided wp/bp inputs, so the device program is rank-independent).
    No ReduceScatter, no fp32 partial traffic, no output copies: c_proj
    results go straight to the output tensor.

Device algorithm (per core), matmuls bf16 with fp32 PSUM accumulation:
  xT (C,T) staged transposed by host; inputs staged in dependency order so
  the first qkv matmuls start ~9us in.
  qT = wq^T @ xT, kT = wk^T @ xT   (feature-major, 512-token units)
  v  = x @ wv                      (token-major) + ones column per head
  per head pair (2fc, 2fc+1), per q-block Q (512 wide):
    s^T[kchunk] = kT_h^T @ qT_h    (K=64 contraction, row-tiled pair ->
        concurrent); columns trimmed to the causal range on diagonal chunks.
    p = exp(0.125 * s^T)  (ScalarE, bf16 out) over causal columns only; the
        128-wide diagonal triangle is zeroed by a DVE multiply with a
        precomputed triangular mask (GpSimd affine_select builds it once).
    o^T[65,W] += v_aug_h^T @ p     (ones column -> row 64 = softmax denom)
    r = approx_recip(denoms) on DVE; r is partition-broadcast on GpSimd
    (daisy chain, SBUF only -- no DMA round trip, no PSUM); y_h = o^T * bc,
    staged to DRAM and AllGathered across the pair. The broadcast +
    multiplies + exchange are deferred into the next head-pair's score
    stream so their waits never head-block a queue.
  c_proj pipelined one q-block behind, one 128-token block per head-pair
  slot, interleaved into the exp-bound attention phase as PE filler; its
  contraction orders the last-exchanged feature chunks last so it can start
  before the final AllGather lands.
"""

import sys

if "/opt/trn_rl_repo" not in sys.path:
    sys.path.insert(0, "/opt/trn_rl_repo")

import numpy as np
import ml_dtypes

import concourse.bass as bass
import concourse.bacc as bacc
import concourse.mybir as mybir
import concourse.tile as tile
from concourse.bass import ts, ds
from concourse.bass_utils import run_bass_kernel_spmd

BF16 = ml_dtypes.bfloat16
N_CORES = 8
B, T, C = 4, 2048, 1024
NH, HS = 16, 64
H_LOC = NH // 2        # heads per core
F = H_LOC * HS         # 512 local qkv features
NFC = F // 128         # 4 feature chunks (one head pair each)
NKC = T // 128         # 16 key chunks
NQ = T // 512          # 4 query blocks
KO = C // 128          # 8 contraction chunks for c_proj (full features)
REPLICA_GROUPS = [[0, 1], [2, 3], [4, 5], [6, 7]]
# c_proj contraction order: chunks from the last-exchanged head pairs last,
# so the token-block matmuls can start before the final AllGather lands.
# Blocks 0-2 process head pairs in order 0..3; block 3 runs [3,0,1,2] so its
# final exchange (head pair 2 -> global chunks 2 and 6) happens mid-block.
KO_ORDER = [0, 4, 1, 5, 2, 6, 3, 7]
KO_TAIL = [0, 4, 1, 5, 3, 7, 2, 6]

FP32 = mybir.dt.float32
BF = mybir.dt.bfloat16


def _build_nc():
    # Bacc (not plain Bass): its compile() pipeline runs
    # generate_event_semaphores, which splits sync waits so no instruction
    # carries more than the hardware allows (walrus rejects >1 otherwise).
    nc = bacc.Bacc(None, target_bir_lowering=False, num_devices=N_CORES)

    xT = nc.dram_tensor("xT", [C, T], BF, kind="ExternalInput")
    wq = nc.dram_tensor("wq", [C, F], BF, kind="ExternalInput")
    wk = nc.dram_tensor("wk", [C, F], BF, kind="ExternalInput")
    wv = nc.dram_tensor("wv", [C, F], BF, kind="ExternalInput")
    bq = nc.dram_tensor("bq", [F], FP32, kind="ExternalInput")
    bk = nc.dram_tensor("bk", [F], FP32, kind="ExternalInput")
    bv = nc.dram_tensor("bv", [F], FP32, kind="ExternalInput")
    wp = nc.dram_tensor("wp", [C, F], BF, kind="ExternalInput")  # col slice
    bp = nc.dram_tensor("bp", [F], FP32, kind="ExternalInput")   # col slice
    out = nc.dram_tensor("out", [T, F], FP32, kind="ExternalOutput")

    with tile.TileContext(nc) as tc:
        _body(tc, xT, wq, wk, wv, bq, bk, bv, wp, bp, out)
    nc.compile()
    return nc


def _body(tc, xT, wq, wk, wv, bq, bk, bv, wp, bp, out):
    nc = tc.nc
    import contextlib

    ctx = contextlib.ExitStack()
    with ctx:
        wpool = ctx.enter_context(tc.tile_pool(name="weights", bufs=1))
        apool = ctx.enter_context(tc.tile_pool(name="acts", bufs=1))
        ppool = ctx.enter_context(tc.tile_pool(name="ptiles", bufs=3))
        npool = ctx.enter_context(tc.tile_pool(name="norm", bufs=2))
        yfpool = ctx.enter_context(tc.tile_pool(name="yfull", bufs=3))
        outp = ctx.enter_context(tc.tile_pool(name="outsb", bufs=3))
        # PSUM budget (8 banks): sAB [128,1024] x3 bufs = 6, oA/oB 1 each = 2
        ps_s = ctx.enter_context(tc.tile_pool(name="ps_s", bufs=3, space="PSUM"))
        ps_o = ctx.enter_context(tc.tile_pool(name="ps_o", bufs=1, space="PSUM"))
        dpool = ctx.enter_context(tc.tile_pool(name="dram", bufs=1, space="DRAM"))

        # ---- activation-table preload: a tiny exp up front so the ~2.7us
        # ACT_TABLE_LOAD overlaps input staging instead of the first score.
        warm_in = wpool.tile([1, 16], FP32)
        warm_out = wpool.tile([1, 16], BF)
        nc.vector.memset(warm_in, 0.0)
        nc.scalar.activation(
            out=warm_out, in_=warm_in,
            func=mybir.ActivationFunctionType.Exp, scale=1.0,
        )
        # ... and a tiny AllGather so the ~18us collective-stream bootstrap
        # overlaps staging instead of the first real exchange
        wg_in = dpool.tile([1, 16], BF, name="wg_in")
        wg_out = dpool.tile([2, 1, 16], BF, name="wg_out")
        nc.gpsimd.collective_compute(
            "AllGather",
            mybir.AluOpType.bypass,
            replica_groups=REPLICA_GROUPS,
            ins=[wg_in[:]],
            outs=[wg_out[:]],
        )

        # ---- stage inputs into SBUF (ordered so compute starts early) ----
        wq_sb = wpool.tile([128, KO, F], BF)
        nc.sync.dma_start(out=wq_sb, in_=wq.rearrange("(ko p) f -> p ko f", p=128))
        bq_sb = wpool.tile([128, NFC], FP32)
        nc.sync.dma_start(out=bq_sb, in_=bq.rearrange("(fo p) -> p fo", p=128))
        # x staged in four token quarters so the first qk units start sooner
        x_sb = wpool.tile([128, KO, T], BF)
        x_re = xT.rearrange("(ko p) t -> p ko t", p=128)
        nc.sync.dma_start(out=x_sb[:, :, 0:512], in_=x_re[:, :, 0:512])
        wk_sb = wpool.tile([128, KO, F], BF)
        nc.sync.dma_start(out=wk_sb, in_=wk.rearrange("(ko p) f -> p ko f", p=128))
        bk_sb = wpool.tile([128, NFC], FP32)
        nc.sync.dma_start(out=bk_sb, in_=bk.rearrange("(fo p) -> p fo", p=128))
        wv_sb = wpool.tile([128, KO, F], BF)
        nc.sync.dma_start(out=wv_sb, in_=wv.rearrange("(ko p) f -> p ko f", p=128))
        # broadcast biases across partitions (for token-major layouts);
        # early: v_unit's bias add must not head-block the Vector queue
        bv_bc = wpool.tile([128, F], FP32)
        nc.sync.dma_start(
            out=bv_bc,
            in_=bass.AP(tensor=bv.ap().tensor, offset=0, ap=[[0, 128], [1, F]]),
        )
        nc.sync.dma_start(out=x_sb[:, :, 512:1024], in_=x_re[:, :, 512:1024])
        nc.sync.dma_start(out=x_sb[:, :, 1024:1536], in_=x_re[:, :, 1024:1536])
        nc.sync.dma_start(out=x_sb[:, :, 1536:2048], in_=x_re[:, :, 1536:2048])
        wp_sb = wpool.tile([128, KO, F], BF)
        nc.sync.dma_start(out=wp_sb, in_=wp.rearrange("(ko p) n -> p ko n", p=128))
        bp_bc = wpool.tile([128, F], FP32)
        nc.sync.dma_start(
            out=bp_bc,
            in_=bass.AP(tensor=bp.ap().tensor, offset=0, ap=[[0, 128], [1, F]]),
        )

        # ---- constants ----
        # triangular causal mask for the 128-wide diagonal band:
        # tri[p, h, t] = 1 if t >= p else 0  (query-offset t vs key p)
        tri = wpool.tile([128, 2, 128], BF)
        nc.vector.memset(tri, 1.0)
        nc.gpsimd.affine_select(
            out=tri, in_=tri,
            compare_op=mybir.AluOpType.is_ge,
            fill=0.0, base=0, channel_multiplier=-1,
            pattern=[[0, 2], [1, 128]],
        )

        # ---- persistent activations ----
        qT_sb = apool.tile([128, NFC, T], BF)   # q, feature-major
        kT_sb = apool.tile([128, NFC, T], BF)   # k, feature-major
        # v token-major, 66-stride per head: cols 0:64 = v, col 64 = ones
        v_sb = apool.tile([128, NKC, H_LOC, 66], BF)
        nc.vector.memset(v_sb[:, :, :, 64:65], 1.0)

        # per-(block, head-pair) AllGather staging. yd = our 128-feature
        # chunk of y^T for the block; ya[r] = rank r's chunk (rank 0 = head
        # group 0 = global feature chunk fc, rank 1 = chunk 4+fc).
        yd = [[dpool.tile([128, 512], BF, name=f"yd{q}_{f}") for f in range(NFC)]
              for q in range(NQ)]
        ya = [[dpool.tile([2, 128, 512], BF, name=f"ya{q}_{f}") for f in range(NFC)]
              for q in range(NQ)]
        yfs = {}  # Q -> gathered full-feature y^T [128, KO, 512] in SBUF

        # ---- qkv projection units (512-token granularity so they slot
        # finely into the attention phase as PE filler) ----
        def qk_unit(w_sb, b_sb, dst, fc, tq):
            ps = ps_s.tile([128, 1024], FP32, tag="sAB")
            for kc in range(KO):
                nc.tensor.matmul(
                    ps[:, 0:512],
                    lhsT=w_sb[:, kc, ts(fc, 128)],
                    rhs=x_sb[:, kc, ts(tq, 512)],
                    start=(kc == 0),
                    stop=(kc == KO - 1),
                )
            nc.scalar.activation(
                out=dst[:, fc, ts(tq, 512)],
                in_=ps[:, 0:512],
                func=mybir.ActivationFunctionType.Identity,
                bias=b_sb[:, fc : fc + 1],
                scale=1.0,
            )

        def v_unit(tc_i):
            ps = ps_s.tile([128, 1024], FP32, tag="sAB")
            for kc in range(KO):
                nc.tensor.matmul(
                    ps[:, 0:512],
                    lhsT=x_sb[:, kc, ts(tc_i, 128)],
                    rhs=wv_sb[:, kc, :],
                    start=(kc == 0),
                    stop=(kc == KO - 1),
                )
            nc.vector.tensor_add(
                out=v_sb[:, tc_i, :, 0:64],
                in0=ps[:, 0:512].rearrange("p (h f) -> p h f", h=H_LOC),
                in1=bv_bc.rearrange("p (h f) -> p h f", h=H_LOC),
            )

        # ---- c_proj: our 512 output columns for one 128-token block,
        # split so the chunks fed by the block's last AllGather (head pair
        # 3 -> global chunks 3 and 7) can be emitted separately ----
        def proj_tb_begin(Q, tb, order=KO_ORDER):
            yf = yfs[Q]
            ps = ps_s.tile([128, 1024], FP32, tag="sAB")
            for ko in order[:-2]:
                nc.tensor.matmul(
                    ps[:, 0:512],
                    lhsT=yf[:, ko, ts(tb, 128)],
                    rhs=wp_sb[:, ko, :],
                    start=(ko == order[0]),
                    stop=False,
                )
            return ps

        def proj_tb_end(Q, tb, ps, order=KO_ORDER):
            yf = yfs[Q]
            for ko in order[-2:]:
                nc.tensor.matmul(
                    ps[:, 0:512],
                    lhsT=yf[:, ko, ts(tb, 128)],
                    rhs=wp_sb[:, ko, :],
                    start=False,
                    stop=(ko == order[-1]),
                )
            o_sb = outp.tile([128, 512], FP32, tag="osb")
            nc.vector.tensor_add(out=o_sb, in0=ps[:, 0:512], in1=bp_bc)
            nc.sync.dma_start(
                out=out.ap()[ds(Q * 512 + tb * 128, 128), :], in_=o_sb
            )

        def proj_tb(Q, tb):
            proj_tb_end(Q, tb, proj_tb_begin(Q, tb))

        # ---- attention ----
        pending = []  # deferred normalization phase-2 closures
        pending_reload = []  # last-block reloads, deferred one more flush so
        # their AllGather wait is already over when they hit the queue

        def flush_pending():
            # old reloads first: their AllGathers are a full flush old, so
            # they dequeue without waiting and never head-block a doorbell
            while pending_reload:
                pending_reload.pop(0)()
            while pending:
                pending.pop(0)()

        def attention_block(Q, pre_fc=None, slot_fns=None, fc_order=None):
            pre_fc = pre_fc or {}
            slot_fns = slot_fns or {}
            nkc = 4 * Q + 4  # causal: only key chunks 0 .. 4Q+3 contribute
            LAG = 2  # AV matmuls trail the QK/exp pipeline by this many chunks
            yf = yfpool.tile([128, KO, 512], BF, tag="yf")
            yfs[Q] = yf
            for pos, fc in enumerate(fc_order or range(NFC)):
                for u in pre_fc.get(pos, ()):
                    u()
                # spread this head-pair's filler work (qkv units / c_proj
                # blocks) evenly through the chunk loop: a ~2us unit is
                # absorbed by the 2-chunk exp backlog, while a single big
                # burst at the boundary starves the exp pipeline
                fns = list(slot_fns.get(pos, ()))
                spots = {}
                for i_f in range(len(fns)):
                    pos = min(nkc - 1, (i_f + 1) * nkc // (len(fns) + 1))
                    spots.setdefault(pos, []).append(fns[i_f])
                to = ps_o.tile([128, 512], FP32, tag="oA")
                tb_ = ps_o.tile([128, 512], FP32, tag="oB")
                pbuf = {}

                def emit_av(kc, to=to, tb_=tb_, nkc=nkc, fc=fc):
                    pAB, q0 = pbuf.pop(kc)
                    w = 512 - q0
                    nc.tensor.matmul(
                        to[0:65, ds(q0, w)],
                        lhsT=v_sb[:, kc, 2 * fc, 0:65],
                        rhs=pAB[:, ds(q0, w)],
                        start=(kc == 0),
                        stop=(kc == nkc - 1),
                    )
                    nc.tensor.matmul(
                        tb_[0:65, ds(q0, w)],
                        lhsT=v_sb[:, kc, 2 * fc + 1, 0:65],
                        rhs=pAB[:, ds(512 + q0, w)],
                        start=(kc == 0),
                        stop=(kc == nkc - 1),
                    )

                for kc in range(nkc):
                    j = kc - 4 * Q  # >= 0 on the diagonal band
                    q0 = 128 * j if j > 0 else 0
                    w = 512 - q0
                    # heads A and B share one 2-bank psum tile: A in cols
                    # 0:512 (array rows 0:64), B in 512:1024 (rows 64:128);
                    # the row-tiled pair runs concurrently on the PE.
                    sAB = ps_s.tile([128, 1024], FP32, tag="sAB")
                    nc.tensor.matmul(
                        sAB[:, ds(q0, w)],
                        lhsT=kT_sb[0:64, fc, ts(kc, 128)],
                        rhs=qT_sb[0:64, fc, ds(Q * 512 + q0, w)],
                        start=True,
                        stop=True,
                        tile_position=(0, 0),
                    )
                    nc.tensor.matmul(
                        sAB[:, ds(512 + q0, w)],
                        lhsT=kT_sb[64:128, fc, ts(kc, 128)],
                        rhs=qT_sb[64:128, fc, ds(Q * 512 + q0, w)],
                        start=True,
                        stop=True,
                        tile_position=(64, 0),
                    )
                    if kc == 2:
                        flush_pending()
                    pAB = ppool.tile([128, 1024], BF, tag="pAB", bufs=4)
                    pABh = pAB.rearrange("p (h q) -> p h q", h=2)
                    sABh = sAB.rearrange("p (h q) -> p h q", h=2)
                    # exp only the causal columns (columns < q0 are never
                    # read downstream: the AV rhs is trimmed to match)
                    nc.scalar.activation(
                        out=pABh[:, :, ds(q0, w)],
                        in_=sABh[:, :, ds(q0, w)],
                        func=mybir.ActivationFunctionType.Exp,
                        scale=0.125,
                    )
                    if j >= 0:
                        # zero the 128-wide causal triangle (DVE multiply
                        # with the precomputed mask)
                        nc.vector.tensor_mul(
                            out=pABh[:, :, ds(q0, 128)],
                            in0=pABh[:, :, ds(q0, 128)],
                            in1=tri,
                        )
                    pbuf[kc] = (pAB, q0)
                    if kc >= LAG:
                        emit_av(kc - LAG)
                    for fn in spots.get(kc, ()):
                        fn()
                for kc in range(max(0, nkc - LAG), nkc):
                    emit_av(kc)

                # normalization phase 1 (DVE): copy o out of PSUM (freeing
                # the banks), stage the denominator rows to partition 0,
                # approx-reciprocal.
                oA_sb = npool.tile([65, 512], FP32, tag="oAsb")
                oB_sb = npool.tile([65, 512], FP32, tag="oBsb")
                nc.vector.tensor_copy(out=oA_sb, in_=to[0:65, :])
                nc.vector.tensor_copy(out=oB_sb, in_=tb_[0:65, :])
                rz = npool.tile([1, 1024], FP32, tag="rz")
                nc.vector.tensor_copy(out=rz[:, 0:512], in_=oA_sb[64:65, :])
                nc.vector.tensor_copy(out=rz[:, 512:1024], in_=oB_sb[64:65, :])
                r = npool.tile([1, 1024], FP32, tag="r")
                nc.vector.reciprocal_approx_fast(out=r, in_=rz)

                # phase 2 (GpSimd broadcast + DVE multiplies + exchange) is
                # deferred into the next head-pair's score stream so its
                # reciprocal wait never head-blocks the DVE/GpSimd queues.
                def phase2(Q=Q, fc=fc, oA_sb=oA_sb, oB_sb=oB_sb, r=r, yf=yf):
                    # broadcast r from partition 0 to 64 partitions via the
                    # GpSimd daisy chain -- no DMA round trip, no PSUM
                    bc = npool.tile([64, 1024], FP32, tag="bc")
                    nc.gpsimd.partition_broadcast(out_ap=bc, in_ap=r, channels=64)
                    ystA = npool.tile([64, 512], BF, tag="ystA")
                    ystB = npool.tile([64, 512], BF, tag="ystB")
                    nc.vector.tensor_mul(out=ystA, in0=oA_sb[0:64, :], in1=bc[:, 0:512])
                    nc.vector.tensor_mul(out=ystB, in0=oB_sb[0:64, :], in1=bc[:, 512:1024])
                    # stage our feature chunk to DRAM and exchange it
                    ydt = yd[Q][fc]
                    nc.sync.dma_start(out=ydt[ds(0, 64), :], in_=ystA)
                    nc.sync.dma_start(out=ydt[ds(64, 64), :], in_=ystB)
                    nc.gpsimd.collective_compute(
                        "AllGather",
                        mybir.AluOpType.bypass,
                        replica_groups=REPLICA_GROUPS,
                        ins=[ydt[:]],
                        outs=[ya[Q][fc][:]],
                    )
                    # reloads ride the GpSimd SWDGE queue: their AllGather
                    # wait must not head-block the Sync staging DMAs. In the
                    # last block they are deferred one further flush so they
                    # also never delay the next AllGather's doorbell.
                    def reload(Q=Q, fc=fc, yf=yf):
                        nc.gpsimd.dma_start(out=yf[:, fc, :], in_=ya[Q][fc][0])
                        nc.gpsimd.dma_start(out=yf[:, 4 + fc, :], in_=ya[Q][fc][1])

                    if Q == NQ - 1:
                        pending_reload.append(reload)
                    else:
                        reload()

                pending.append(phase2)

        # ---- software-pipelined schedule ----
        # prefix: exactly what attention(0) fc0 needs
        qk_unit(wq_sb, bq_sb, qT_sb, 0, 0)
        qk_unit(wk_sb, bk_sb, kT_sb, 0, 0)
        for i in range(4):
            v_unit(i)

        def qkq(fc, tq):
            return lambda: qk_unit(wq_sb, bq_sb, qT_sb, fc, tq)

        def qkk(fc, tq):
            return lambda: qk_unit(wk_sb, bk_sb, kT_sb, fc, tq)

        attention_block(
            0,
            pre_fc={f: [qkq(f, 0), qkk(f, 0)] for f in (1, 2, 3)},
            slot_fns={f: [qkq(f, 1), qkk(f, 1), (lambda i=f: v_unit(4 + i))]
                      for f in range(4)},
        )
        attention_block(
            1,
            slot_fns={
                f: [qkq(f, 2), qkk(f, 2), (lambda i=f: v_unit(8 + i)),
                    (lambda i=f: proj_tb(0, i))]
                for f in range(4)
            },
        )
        attention_block(
            2,
            slot_fns={
                f: [qkq(f, 3), qkk(f, 3), (lambda i=f: v_unit(12 + i))]
                for f in range(4)
            },
        )
        # block 3 is otherwise exp-bound with an idle (HAM-cooling) PE --
        # both pending c_proj blocks ride there as filler
        attention_block(
            3,
            slot_fns={
                0: [lambda: proj_tb(1, 0), lambda: proj_tb(1, 1)],
                1: [lambda: proj_tb(1, 2), lambda: proj_tb(1, 3)],
                2: [lambda: proj_tb(2, 0), lambda: proj_tb(2, 1)],
                3: [lambda: proj_tb(2, 2), lambda: proj_tb(2, 3)],
            },
            fc_order=[3, 0, 1, 2],
        )
        flush_pending()
        # tail: interleave the four token blocks' early chunks (served by
        # already-landed AllGathers) so they overlap the final exchange
        ps0 = proj_tb_begin(3, 0, KO_TAIL)
        ps1 = proj_tb_begin(3, 1, KO_TAIL)
        ps2 = proj_tb_begin(3, 2, KO_TAIL)
        while pending_reload:  # the final exchange's reload
            pending_reload.pop(0)()
        proj_tb_end(3, 0, ps0, KO_TAIL)
        ps3 = proj_tb_begin(3, 3, KO_TAIL)
        proj_tb_end(3, 1, ps1, KO_TAIL)
        proj_tb_end(3, 2, ps2, KO_TAIL)
        proj_tb_end(3, 3, ps3, KO_TAIL)


_NC_CACHE = None


def _get_nc():
    global _NC_CACHE
    if _NC_CACHE is None:
        _NC_CACHE = _build_nc()
    return _NC_CACHE


def kernel(x, w_attn, b_attn, w_proj, b_proj):
    x = np.asarray(x)
    w_attn = np.asarray(w_attn)
    b_attn = np.asarray(b_attn)
    w_proj = np.asarray(w_proj)
    b_proj = np.asarray(b_proj)

    nc = _get_nc()

    in_maps = []
    for i in range(N_CORES):
        b, g = i // 2, i % 2
        in_maps.append(
            {
                "xT": np.ascontiguousarray(x[b].T).astype(BF16),
                "wq": np.ascontiguousarray(w_attn[:, g * F : (g + 1) * F]).astype(BF16),
                "wk": np.ascontiguousarray(
                    w_attn[:, C + g * F : C + (g + 1) * F]
                ).astype(BF16),
                "wv": np.ascontiguousarray(
                    w_attn[:, 2 * C + g * F : 2 * C + (g + 1) * F]
                ).astype(BF16),
                "bq": np.ascontiguousarray(b_attn[g * F : (g + 1) * F]).astype(
                    np.float32
                ),
                "bk": np.ascontiguousarray(b_attn[C + g * F : C + (g + 1) * F]).astype(
                    np.float32
                ),
                "bv": np.ascontiguousarray(
                    b_attn[2 * C + g * F : 2 * C + (g + 1) * F]
                ).astype(np.float32),
                # column-parallel c_proj: full rows, our 512 output columns
                "wp": np.ascontiguousarray(w_proj[:, g * F : (g + 1) * F]).astype(BF16),
                "bp": np.ascontiguousarray(b_proj[g * F : (g + 1) * F]).astype(
                    np.float32
                ),
            }
        )

    global _last_in_maps
    _last_in_maps = in_maps  # stashed for external profiling harnesses
    res = run_bass_kernel_spmd(nc, in_maps, core_ids=list(range(N_CORES)))

    # Each core's "out" is [T, 512]: all tokens, its 512 output columns.
    out = np.empty((B, T, C), dtype=np.float32)
    for b in range(B):
        out[b][:, 0:F] = res.results[2 * b]["out"]
        out[b][:, F:C] = res.results[2 * b + 1]["out"]
    return out
